# revision 25
# baseline (speedup 1.0000x reference)
"""AMCRNet RoI extractor as a Trainium2 Bass/Tile kernel, data-parallel over
the 8 clips (one clip per NeuronCore).

Math notes (derived from the reference):
  - trans_feat = concat(slow, rearranged fast) -> temporal avg-pool (4->1)
    -> 1x1x1 conv (2048->1024) -> GroupNorm(16) -> ReLU -> (c t) fold.
    The avg-pool's 1/4 is folded into the conv weight host-side, so the
    device only does sums of 4 temporal slices.
  - RoIAlign output is immediately averaged over all 256 output pixels, so
    the sparse RoIAlign map collapses to one weight vector per RoI:
      wbar[n, q] = mean_p Wmat[n, p, q]   (q indexes the 16x16 feature map)
    roi_feat[n, c] = sum_q feat[c, q] * wbar[n, q]  -> small matmuls.
  - rois are bucketed per clip (16 per clip, sorted), so wbar rows shard
    alongside the feature maps.
"""

import sys

sys.path.insert(0, "/opt/trn_rl_repo")

import numpy as np
import ml_dtypes

BF16 = ml_dtypes.bfloat16

# problem constants (hardcoded per spec)
B = 8
N_PER = 16
C_SLOW, T_SLOW = 1024, 8
C_FAST, T_FAST = 256, 32
HF, WF = 16, 16
P_PIX = HF * WF  # 256
C_CAT = 2048
C_OUT = 1024
T_POOL = 2
C_FEAT = C_OUT * T_POOL  # 2048
OUT_SIZE = 16
RATIO = 2
SCALE = 1.0 / 16.0
GN_GROUPS = 16
EPS = 1e-5

_NC_CACHE = {}


def _build_nc():
    import concourse.bacc as bacc
    import concourse.tile as tile
    from concourse import mybir
    from concourse import bass_isa

    f32 = mybir.dt.float32
    bf16 = mybir.dt.bfloat16
    Relu = mybir.ActivationFunctionType.Relu
    Sqrt = mybir.ActivationFunctionType.Sqrt

    nc = bacc.Bacc("TRN2", target_bir_lowering=False, debug=False)

    # ---- per-core DRAM parameters (slow/fast pre-cast to bf16 on host) ----
    slow_d = nc.dram_tensor("slow", [1024, 2048], bf16, kind="ExternalInput")
    fast_d = nc.dram_tensor("fast", [256, 8192], bf16, kind="ExternalInput")
    convT_d = nc.dram_tensor("convT", [2048, 1024], bf16, kind="ExternalInput")
    posT_d = nc.dram_tensor("posT", [256, 2048], bf16, kind="ExternalInput")
    wbarT_d = nc.dram_tensor("wbarT", [256, 16], bf16, kind="ExternalInput")
    gamma_d = nc.dram_tensor("gamma8", [128, 8], f32, kind="ExternalInput")
    beta_d = nc.dram_tensor("beta8", [128, 8], f32, kind="ExternalInput")
    ident_d = nc.dram_tensor("ident", [128, 128], bf16, kind="ExternalInput")

    feat_d = nc.dram_tensor("feat", [2048, 256], f32, kind="ExternalOutput")
    rfeat_d = nc.dram_tensor("roi_feat", [16, 2048], f32, kind="ExternalOutput")
    rpos_d = nc.dram_tensor("roi_pos", [16, 2048], f32, kind="ExternalOutput")

    with tile.TileContext(nc) as tc:
        with (
            tc.tile_pool(name="const", bufs=1) as constp,
            tc.tile_pool(name="sraw", bufs=4) as srawp,
            tc.tile_pool(name="fraw", bufs=4) as frawp,
            tc.tile_pool(name="pooledS", bufs=8) as poolSp,
            tc.tile_pool(name="pooledF", bufs=2) as poolFp,
            tc.tile_pool(name="work", bufs=1) as workp,
            tc.tile_pool(name="outb", bufs=4) as outp,
            tc.tile_pool(name="psum", bufs=8, space="PSUM") as psump,
        ):
            # --------- tiny prep: eps + preload the sqrt/relu ACT table ------
            eps_sb = constp.tile([128, 1], f32, tag="eps")
            nc.vector.memset(eps_sb[:], EPS)
            dummy_sb = workp.tile([128, 1], f32, tag="dummy")
            nc.scalar.activation(dummy_sb[:], eps_sb[:], func=Sqrt, bias=eps_sb[:], scale=1.0)

            # --------- small consts + weights on the scalar HWDGE ring -------
            # (the sync ring is reserved for the big slow/fast input streams)
            wbarT_sb = constp.tile([128, 2, 16], bf16, tag="wbarT")
            for h in range(2):
                nc.scalar.dma_start(wbarT_sb[:, h, :], wbarT_d[h * 128:(h + 1) * 128, :])
            gamma_sb = constp.tile([128, 8], f32, tag="gamma")
            nc.scalar.dma_start(gamma_sb[:], gamma_d[:])
            beta_sb = constp.tile([128, 8], f32, tag="beta")
            nc.scalar.dma_start(beta_sb[:], beta_d[:])
            ident_sb = constp.tile([128, 128], bf16, tag="ident")
            nc.scalar.dma_start(ident_sb[:], ident_d[:])

            posT_sb = constp.tile([128, 2, 2048], bf16, tag="posT")
            for h in range(2):
                nc.scalar.dma_start(posT_sb[:, h, :], posT_d[h * 128:(h + 1) * 128, :])
            convT_sb = constp.tile([128, 16, 1024], bf16, tag="convT")
            convTv = convT_d[:].rearrange("(c p) o -> p c o", p=128)

            # --------- input streams on the sync ring, consumption order -----
            fraw = [
                frawp.tile([128, 4096], bf16, tag="fraw", name=f"fraw{i}")
                for i in range(4)
            ]  # piece (h, t) at index 2*h + t
            sraw = [
                srawp.tile([128, 2, 2048], bf16, tag="sraw", name=f"sraw{i}")
                for i in range(3)
            ]
            sraw67 = [
                srawp.tile([128, 2048], bf16, tag="sraw67", name=f"sraw67_{i}", bufs=2)
                for i in range(2)
            ]
            slowv = slow_d[:].rearrange("(u y p) c -> u p y c", u=4, y=2)

            def dma_fast(h, t):
                nc.sync.dma_start(
                    fraw[2 * h + t][:],
                    fast_d[h * 128:(h + 1) * 128, t * 4096:(t + 1) * 4096],
                )

            nc.sync.dma_start(convT_sb[:, 0:4, :], convTv[:, 0:4, :])
            nc.sync.dma_start(sraw[0][:], slowv[0])
            dma_fast(0, 0)
            dma_fast(0, 1)
            nc.sync.dma_start(convT_sb[:, 8:12, :], convTv[:, 8:12, :])
            nc.sync.dma_start(sraw[1][:], slowv[1])
            dma_fast(1, 0)
            nc.sync.dma_start(convT_sb[:, 12:16, :], convTv[:, 12:16, :])
            dma_fast(1, 1)
            nc.sync.dma_start(convT_sb[:, 4:8, :], convTv[:, 4:8, :])
            nc.sync.dma_start(sraw[2][:], slowv[2])
            nc.sync.dma_start(sraw67[0][:], slow_d[768:896, :])
            nc.sync.dma_start(sraw67[1][:], slow_d[896:1024, :])

            # --------- temporal pooling (sum of 4; x0.25 folded in convT) ----
            # pooled chunk layout: 0..7 slow, 8+2r+h fast
            pooled_rhs = [None] * 16
            pf = [
                poolFp.tile([128, 2, 4, 256], bf16, tag="pf", name=f"pf{i}")
                for i in range(2)
            ]

            def do_fast(h, t):
                v = fraw[2 * h + t][:].rearrange(
                    "p (a b r w) -> p a b r w", a=2, b=2, r=4, w=256
                )
                tmpf = workp.tile([128, 2, 4, 256], bf16, tag="ftmp", bufs=2)
                nc.vector.tensor_add(tmpf[:], v[:, 0], v[:, 1])
                nc.vector.tensor_add(pf[h][:, t], tmpf[:, 0], tmpf[:, 1])

            def do_slow(k):
                s_ap = sraw67[k - 6][:] if k >= 6 else sraw[k // 2][:, k % 2, :]
                v = s_ap.rearrange(
                    "p (t a b w) -> p t a b w", t=2, a=2, b=2, w=256
                )
                eng = nc.gpsimd if k in (2, 3, 4) else nc.vector
                tmp = workp.tile([128, 2, 2, 256], bf16, tag="stmp", bufs=3)
                eng.tensor_add(tmp[:], v[:, :, 0], v[:, :, 1])
                pk = poolSp.tile([128, 2, 256], bf16, tag="ps_slow")
                eng.tensor_add(pk[:], tmp[:, :, 0], tmp[:, :, 1])
                pooled_rhs[k] = pk

            do_slow(0)
            do_slow(1)
            do_fast(0, 0)
            do_fast(0, 1)
            do_slow(2)
            do_slow(3)
            do_fast(1, 0)
            do_fast(1, 1)
            for k in range(4, 8):
                do_slow(k)
            for r in range(4):
                for h in range(2):
                    pooled_rhs[8 + 2 * r + h] = ("fast", h, r)

            # --------- roi_pos fills the PE while inputs stream in -----------
            rpos_sb = constp.tile([16, 2048], f32, tag="rpos")
            # 4-way column-tiled: the four output slices run concurrently in
            # different 32-column groups of the PE array
            rp_all = psump.tile([128, 512], f32, tag="ps", name="rp_all")

            def rpos_mms(h):
                for n in range(4):
                    nc.tensor.matmul(
                        rp_all[32 * n:32 * n + 16, :],
                        lhsT=wbarT_sb[:, h, :],
                        rhs=posT_sb[:, h, n * 512:(n + 1) * 512],
                        start=(h == 0),
                        stop=(h == 1),
                        tile_position=(0, 32 * n),
                    )

            rpos_mms(0)

            # --------- conv matmuls, K-major, ordered by expected arrival ----
            # two M-waves: wave A (m0-4) is DMA-paced; wave B (m5-7) runs dense
            # afterwards while wave A normalizes -> most of the GN/act tail
            # overlaps wave B's matmuls.
            ypsum = [psump.tile([128, 512], f32, tag="ps", name=f"y{i}") for i in range(8)]
            k_order = [0, 1, 2, 3, 8, 10, 12, 14, 9, 11, 13, 15, 4, 5, 6, 7]

            def rhs_of(k):
                pr = pooled_rhs[k]
                if isinstance(pr, tuple):
                    _, h, r = pr
                    return pf[h][:, :, r, :]
                return pr[:]

            def conv_wave(ms):
                first, last = [], None
                for idx, k in enumerate(k_order):
                    rhs = rhs_of(k)
                    for m in ms:
                        inst = nc.tensor.matmul(
                            ypsum[m][:],
                            lhsT=convT_sb[:, k, m * 128:(m + 1) * 128],
                            rhs=rhs,
                            start=(idx == 0),
                            stop=(idx == 15),
                        )
                        if idx == 0:
                            first.append(inst)
                        last = inst
                return first, last

            WAVE_A = [0, 1, 2, 3, 4]
            WAVE_B1 = [5]
            WAVE_B2 = [6, 7]
            _, waveA_last = conv_wave(WAVE_A)
            rpos_mms(1)
            for n in range(4):
                nc.scalar.copy(rpos_sb[:, n * 512:(n + 1) * 512], rp_all[32 * n:32 * n + 16, :])
            nc.scalar.dma_start(rpos_d[:], rpos_sb[:])

            # --------- GroupNorm + ReLU + outputs, one round per wave --------
            # (groups of channel 128m+p are 2m + p//64: each 128-channel tile
            # is self-contained, so wave A normalizes while wave B matmuls)
            featv = feat_d[:].rearrange("(m p t) w -> m p (t w)", m=8, p=128, t=2)
            # featT2 physical layout: [q, h, k, (t, p)] so each transposed pair
            # lands with one contiguous copy; the roi matmul re-orders via AP.
            featT_sb = constp.tile([128, 2, 8, 256], bf16, tag="featT")
            rfeat_sb = constp.tile([16, 2048], f32, tag="rfeat")

            stats3 = workp.tile([128, 8, 3], f32, tag="stats3")
            nmu = workp.tile([128, 8], f32, tag="nmu")
            e2s = workp.tile([128, 8], f32, tag="e2s")
            musq = workp.tile([128, 8], f32, tag="musq")
            var = workp.tile([128, 8], f32, tag="var")
            std = workp.tile([128, 8], f32, tag="std")
            rstd = workp.tile([128, 8], f32, tag="rstd")
            scl = workp.tile([128, 8], f32, tag="scl")
            bias = workp.tile([128, 8], f32, tag="bias")

            def gn_stats(ms):
                for m in ms:
                    bnst = workp.tile([128, 6], f32, tag="bnst", bufs=2)
                    nc.vector.bn_stats(bnst[:], ypsum[m][:])
                    nc.vector.bn_aggr(stats3[:, m, 0:2], bnst[:])
                    nc.vector.tensor_mul(
                        stats3[:, m, 2:3], stats3[:, m, 0:1], stats3[:, m, 0:1]
                    )

            def gn_finish_from(ms, bcs):
                lo, hi = ms[0], ms[-1] + 1
                nm = hi - lo
                sl = slice(lo, hi)
                b3 = bcs[:, 0:3 * nm].rearrange("p (m s) -> p m s", m=nm)
                nc.vector.tensor_scalar_mul(nmu[:, sl], b3[:, :, 0], -1.0 / 64.0)
                nc.vector.tensor_add(e2s[:, sl], b3[:, :, 1], b3[:, :, 2])
                nc.vector.tensor_mul(musq[:, sl], nmu[:, sl], nmu[:, sl])
                nc.vector.tensor_scalar_mul(var[:, sl], e2s[:, sl], 1.0 / 64.0)
                nc.vector.tensor_sub(var[:, sl], var[:, sl], musq[:, sl])
                nc.scalar.activation(std[:, sl], var[:, sl], func=Sqrt, bias=eps_sb[:], scale=1.0)
                nc.vector.reciprocal(rstd[:, sl], std[:, sl])
                nc.vector.tensor_mul(scl[:, sl], gamma_sb[:, sl], rstd[:, sl])
                nc.vector.tensor_mul(bias[:, sl], nmu[:, sl], scl[:, sl])
                nc.vector.tensor_add(bias[:, sl], bias[:, sl], beta_sb[:, sl])

            def act_one(m, on_vector):
                featB = outp.tile([128, 512], bf16, tag="featB", bufs=6)
                if on_vector:
                    nc.vector.tensor_scalar(
                        featB[:], ypsum[m][:], scalar1=scl[:, m:m + 1],
                        scalar2=bias[:, m:m + 1],
                        op0=mybir.AluOpType.mult, op1=mybir.AluOpType.add,
                    )
                    nc.vector.tensor_scalar_max(featB[:], featB[:], 0.0)
                else:
                    nc.scalar.activation(
                        featB[:], ypsum[m][:], func=Relu,
                        bias=bias[:, m:m + 1], scale=scl[:, m:m + 1],
                    )
                # fp32 feat goes out via SWDGE cast-DMA (gpsimd is idle here)
                nc.gpsimd.dma_start(featv[m], featB[:])
                return featB

            def transpose_one(m, featB, copy_vec):
                fBv = featB[:].rearrange("p (t h w) -> p t h w", t=2, h=2, w=128)
                for h in range(2):
                    tp = psump.tile([128, 256], bf16, tag="ps", name=f"tp{m}_{h}")
                    for t in range(2):
                        nc.tensor.matmul(
                            tp[:, t * 128:(t + 1) * 128],
                            lhsT=fBv[:, t, h, :],
                            rhs=ident_sb[:],
                            is_transpose=True,
                            start=True,
                            stop=True,
                        )
                    if (h + (1 if copy_vec else 0)) % 2 == 0:
                        nc.scalar.copy(featT_sb[:, h, m, :], tp[:])
                    else:
                        nc.vector.tensor_copy(featT_sb[:, h, m, :], tp[:])

            def rfeat_pair(n0, cnt=2):
                # roi_feat slices packed into different 32-column groups
                rf = psump.tile([128, 512], f32, tag="ps", name=f"rf{n0}")
                for h in range(2):
                    for j in range(cnt):
                        n = n0 + j
                        rhs = featT_sb[:, h, 2 * n:2 * n + 2, :].rearrange(
                            "q k (t p) -> q k p t", t=2, p=128
                        )
                        nc.tensor.matmul(
                            rf[32 * j:32 * j + 16, :],
                            lhsT=wbarT_sb[:, h, :], rhs=rhs,
                            start=(h == 0), stop=(h == 1),
                            tile_position=(0, 32 * j),
                        )
                for j in range(cnt):
                    n = n0 + j
                    if j == 0:
                        nc.scalar.copy(rfeat_sb[:, n * 512:(n + 1) * 512], rf[32 * j:32 * j + 16, :])
                    else:
                        nc.vector.tensor_copy(rfeat_sb[:, n * 512:(n + 1) * 512], rf[32 * j:32 * j + 16, :])

            # wave A stats; cross-partition sums go through gpsimd so the PE
            # can start wave B with zero stall.  m5 joins GN round A: its stats
            # land right after wave B1 closes, so only m6/m7 remain in the
            # final round.
            gn_stats(WAVE_A)
            waveB1_first, waveB1_last = conv_wave(WAVE_B1)
            for inst in waveB1_first:
                tile.add_dep_helper(inst.ins, waveA_last.ins, reason="wave order")
            gn_stats(WAVE_B1)
            phi = workp.tile([128, 2, 18], f32, tag="phi")
            nc.vector.memset(phi[:], 0.0)
            nc.vector.tensor_copy(
                phi[0:64, 0, :], stats3[0:64, 0:6, :].rearrange("p m s -> p (m s)")
            )
            nc.vector.tensor_copy(
                phi[64:128, 1, :], stats3[64:128, 0:6, :].rearrange("p m s -> p (m s)")
            )
            phi2 = workp.tile([128, 2, 18], f32, tag="phi2")
            nc.gpsimd.partition_all_reduce(
                phi2[:], phi[:], 128, bass_isa.ReduceOp.add
            )
            waveB2_first, _ = conv_wave(WAVE_B2)
            for inst in waveB2_first:
                tile.add_dep_helper(inst.ins, waveB1_last.ins, reason="wave order")
            bcsA = workp.tile([128, 24], f32, tag="bcs", bufs=2)
            nc.vector.tensor_copy(bcsA[0:64, 0:18], phi2[0:64, 0, :])
            nc.vector.tensor_copy(bcsA[64:128, 0:18], phi2[64:128, 1, :])
            gn_finish_from([0, 1, 2, 3, 4, 5], bcsA)
            featBs = {}
            for m in [0, 1, 2, 3, 4, 5]:
                featBs[m] = act_one(m, on_vector=(m % 2 == 1))

            # PE after wave B2: round-A transposes + roi slices, then m6/m7
            for m in [0, 1, 2, 3, 4, 5]:
                transpose_one(m, featBs[m], copy_vec=(m % 2 == 0))
                if m in (3, 5):
                    rfeat_pair(m - 3 if m == 3 else 2, 2 if m == 3 else 1)
            gn_stats(WAVE_B2)
            phiB = workp.tile([128, 2, 6], f32, tag="phiB")
            nc.vector.memset(phiB[:], 0.0)
            nc.vector.tensor_copy(
                phiB[0:64, 0, :], stats3[0:64, 6:8, :].rearrange("p m s -> p (m s)")
            )
            nc.vector.tensor_copy(
                phiB[64:128, 1, :], stats3[64:128, 6:8, :].rearrange("p m s -> p (m s)")
            )
            phiB2 = workp.tile([128, 2, 6], f32, tag="phiB2")
            nc.gpsimd.partition_all_reduce(
                phiB2[:], phiB[:], 128, bass_isa.ReduceOp.add
            )
            bcsB = workp.tile([128, 24], f32, tag="bcs", bufs=2)
            nc.vector.tensor_copy(bcsB[0:64, 0:6], phiB2[0:64, 0, :])
            nc.vector.tensor_copy(bcsB[64:128, 0:6], phiB2[64:128, 1, :])
            gn_finish_from([6, 7], bcsB)
            for m in [6, 7]:
                fB = act_one(m, on_vector=(m == 7))
                transpose_one(m, fB, copy_vec=(m % 2 == 0))
                if m == 7:
                    rfeat_pair(3, 1)
            nc.sync.dma_start(rfeat_d[:], rfeat_sb[:])

    nc.compile()
    return nc


def _get_nc():
    if "nc" not in _NC_CACHE:
        _NC_CACHE["nc"] = _build_nc()
    return _NC_CACHE["nc"]


def _roialign_wbar(rois):
    """Mean (over the 16x16 output pixels) RoIAlign weight vector per RoI.

    Numpy port of the reference's roialign_weights followed by mean over P.
    Returns [N, 256] float32.
    """
    rois = np.asarray(rois, np.float32)
    n = rois.shape[0]
    x1 = rois[:, 1] * SCALE - 0.5
    y1 = rois[:, 2] * SCALE - 0.5
    x2 = rois[:, 3] * SCALE - 0.5
    y2 = rois[:, 4] * SCALE - 0.5
    bw = (x2 - x1) / OUT_SIZE
    bh = (y2 - y1) / OUT_SIZE
    grid = (
        np.arange(OUT_SIZE, dtype=np.float32)[:, None]
        + (np.arange(RATIO, dtype=np.float32)[None, :] + 0.5) / RATIO
    )  # [O, r]
    ys = y1[:, None, None] + grid[None] * bh[:, None, None]  # [N, O, r]
    xs = x1[:, None, None] + grid[None] * bw[:, None, None]
    Y = np.broadcast_to(ys[:, :, None, :, None], (n, OUT_SIZE, OUT_SIZE, RATIO, RATIO))
    X = np.broadcast_to(xs[:, None, :, None, :], (n, OUT_SIZE, OUT_SIZE, RATIO, RATIO))
    valid = ((Y >= -1.0) & (Y <= HF) & (X >= -1.0) & (X <= WF)).astype(np.float32)
    y = np.maximum(Y, 0.0)
    x = np.maximum(X, 0.0)
    y0f = np.floor(y)
    x0f = np.floor(x)
    ye = y0f >= HF - 1
    xe = x0f >= WF - 1
    y0 = np.where(ye, HF - 1, y0f).astype(np.int32)
    y1i = np.where(ye, HF - 1, y0f + 1).astype(np.int32)
    x0 = np.where(xe, WF - 1, x0f).astype(np.int32)
    x1i = np.where(xe, WF - 1, x0f + 1).astype(np.int32)
    ly = np.where(ye, 0.0, y - y0f).astype(np.float32)
    lx = np.where(xe, 0.0, x - x0f).astype(np.float32)
    hy = 1.0 - ly
    hx = 1.0 - lx
    cnt = np.float32(RATIO * RATIO)
    w = np.stack([hy * hx, hy * lx, ly * hx, ly * lx], axis=-1) * (valid / cnt)[..., None]
    idx = np.stack(
        [y0 * WF + x0, y0 * WF + x1i, y1i * WF + x0, y1i * WF + x1i], axis=-1
    )
    wbar = np.zeros((n, HF * WF), np.float32)
    flat_idx = idx.reshape(n, -1)
    flat_w = (w / np.float32(OUT_SIZE * OUT_SIZE)).reshape(n, -1)
    np.add.at(wbar, (np.arange(n)[:, None], flat_idx), flat_w)
    return wbar


def _prep_in_maps(slow_feat, fast_feat, rois, pos, conv_w, gn_gamma, gn_beta):
    slow_feat = np.asarray(slow_feat, np.float32)
    fast_feat = np.asarray(fast_feat, np.float32)
    pos = np.asarray(pos, np.float32)
    conv_w = np.asarray(conv_w, np.float32)

    # 1/4 of the temporal mean folded into the conv weight
    convT = np.ascontiguousarray(conv_w.T * np.float32(0.25)).astype(BF16)
    posT = np.ascontiguousarray(pos.reshape(C_FEAT, P_PIX).T).astype(BF16)
    gamma8 = np.ascontiguousarray(
        np.asarray(gn_gamma, np.float32).reshape(8, 128).T
    )
    beta8 = np.ascontiguousarray(np.asarray(gn_beta, np.float32).reshape(8, 128).T)
    ident = np.eye(128, dtype=np.float32).astype(BF16)

    wbar = _roialign_wbar(rois)  # [128, 256]
    # rois are bucketed: N_PER per clip, sorted by batch index (static reshape
    # exactly as in the reference)
    wbarT_all = np.ascontiguousarray(
        wbar.reshape(B, N_PER, P_PIX).transpose(0, 2, 1)
    ).astype(BF16)  # [B, 256, 16]

    in_maps = []
    for b in range(B):
        in_maps.append(
            dict(
                slow=slow_feat[b].reshape(C_SLOW, T_SLOW * P_PIX).astype(BF16),
                fast=fast_feat[b].reshape(C_FAST, T_FAST * P_PIX).astype(BF16),
                convT=convT,
                posT=posT,
                wbarT=np.ascontiguousarray(wbarT_all[b]),
                gamma8=gamma8,
                beta8=beta8,
                ident=ident,
            )
        )
    return in_maps


def _ensure_ntff_hook():
    """Register the axon NTFF profile hook that the boot path skips when the
    image's antenv stub lacks axon_hooks. Test/profiling only."""
    try:
        from antenv.axon_hooks import get_axon_ntff_profile_hook  # noqa: F401
        return
    except ImportError:
        pass
    import types
    import antenv

    if "/root/.axon_site" not in sys.path:
        sys.path.insert(0, "/root/.axon_site")
    from trn_agent_boot.trn_boot import _ntff_profile_via_ctypes

    hook = _ntff_profile_via_ctypes("/opt/axon/libaxon_pjrt.so")
    mod = types.ModuleType("antenv.axon_hooks")
    mod.get_axon_ntff_profile_hook = lambda: hook
    mod.set_axon_ntff_profile_hook = lambda h: None
    sys.modules["antenv.axon_hooks"] = mod
    antenv.axon_hooks = mod

    # artifact upload has no bucket in this container; neuter it
    from concourse import bass_utils

    bass_utils.upload_artifacts = lambda tmpdir: tmpdir


def _run(in_maps, trace=False):
    from concourse.bass_utils import run_bass_kernel_spmd

    if trace:
        _ensure_ntff_hook()
    nc = _get_nc()
    res = run_bass_kernel_spmd(nc, in_maps, core_ids=list(range(B)), trace=trace)
    _NC_CACHE["last_res"] = res
    return res


def _assemble(res):
    feat = np.stack(
        [res.results[b]["feat"].reshape(C_FEAT, HF, WF) for b in range(B)]
    )
    roi_feat = np.stack([res.results[b]["roi_feat"] for b in range(B)])
    roi_pos = np.stack([res.results[b]["roi_pos"] for b in range(B)])
    return feat, roi_feat, roi_pos


def kernel(slow_feat, fast_feat, rois, pos, conv_w, gn_gamma, gn_beta):
    in_maps = _prep_in_maps(slow_feat, fast_feat, rois, pos, conv_w, gn_gamma, gn_beta)
    res = _run(in_maps, trace=False)
    return _assemble(res)


def kernel_traced(slow_feat, fast_feat, rois, pos, conv_w, gn_gamma, gn_beta):
    """Same as kernel() but captures a neuron-profile trace; returns
    (outputs, exec_time_ns)."""
    in_maps = _prep_in_maps(slow_feat, fast_feat, rois, pos, conv_w, gn_gamma, gn_beta)
    res = _run(in_maps, trace=True)
    return _assemble(res), res.exec_time_ns


# revision 26
# speedup vs baseline: 1.0151x; 1.0151x over previous
"""AMCRNet RoI extractor as a Trainium2 Bass/Tile kernel, data-parallel over
the 8 clips (one clip per NeuronCore).

Math notes (derived from the reference):
  - trans_feat = concat(slow, rearranged fast) -> temporal avg-pool (4->1)
    -> 1x1x1 conv (2048->1024) -> GroupNorm(16) -> ReLU -> (c t) fold.
    The avg-pool's 1/4 is folded into the conv weight host-side, so the
    device only does sums of 4 temporal slices.
  - RoIAlign output is immediately averaged over all 256 output pixels, so
    the sparse RoIAlign map collapses to one weight vector per RoI:
      wbar[n, q] = mean_p Wmat[n, p, q]   (q indexes the 16x16 feature map)
    roi_feat[n, c] = sum_q feat[c, q] * wbar[n, q]  -> small matmuls.
  - rois are bucketed per clip (16 per clip, sorted), so wbar rows shard
    alongside the feature maps.
"""

import sys

sys.path.insert(0, "/opt/trn_rl_repo")

import numpy as np
import ml_dtypes

BF16 = ml_dtypes.bfloat16

# problem constants (hardcoded per spec)
B = 8
N_PER = 16
C_SLOW, T_SLOW = 1024, 8
C_FAST, T_FAST = 256, 32
HF, WF = 16, 16
P_PIX = HF * WF  # 256
C_CAT = 2048
C_OUT = 1024
T_POOL = 2
C_FEAT = C_OUT * T_POOL  # 2048
OUT_SIZE = 16
RATIO = 2
SCALE = 1.0 / 16.0
GN_GROUPS = 16
EPS = 1e-5

_NC_CACHE = {}


def _build_nc():
    import concourse.bacc as bacc
    import concourse.tile as tile
    from concourse import mybir
    from concourse import bass_isa

    f32 = mybir.dt.float32
    bf16 = mybir.dt.bfloat16
    Relu = mybir.ActivationFunctionType.Relu
    Sqrt = mybir.ActivationFunctionType.Sqrt

    nc = bacc.Bacc("TRN2", target_bir_lowering=False, debug=False)

    # ---- per-core DRAM parameters (slow/fast pre-cast to bf16 on host) ----
    slow_d = nc.dram_tensor("slow", [1024, 2048], bf16, kind="ExternalInput")
    fast_d = nc.dram_tensor("fast", [256, 8192], bf16, kind="ExternalInput")
    convT_d = nc.dram_tensor("convT", [2048, 1024], bf16, kind="ExternalInput")
    posT_d = nc.dram_tensor("posT", [256, 2048], bf16, kind="ExternalInput")
    wbarT_d = nc.dram_tensor("wbarT", [256, 16], bf16, kind="ExternalInput")
    gamma_d = nc.dram_tensor("gamma8", [128, 8], f32, kind="ExternalInput")
    beta_d = nc.dram_tensor("beta8", [128, 8], f32, kind="ExternalInput")
    ident_d = nc.dram_tensor("ident", [128, 128], bf16, kind="ExternalInput")

    feat_d = nc.dram_tensor("feat", [2048, 256], f32, kind="ExternalOutput")
    rfeat_d = nc.dram_tensor("roi_feat", [16, 2048], f32, kind="ExternalOutput")
    rpos_d = nc.dram_tensor("roi_pos", [16, 2048], f32, kind="ExternalOutput")

    with tile.TileContext(nc) as tc:
        with (
            tc.tile_pool(name="const", bufs=1) as constp,
            tc.tile_pool(name="sraw", bufs=4) as srawp,
            tc.tile_pool(name="fraw", bufs=4) as frawp,
            tc.tile_pool(name="pooledS", bufs=8) as poolSp,
            tc.tile_pool(name="pooledF", bufs=2) as poolFp,
            tc.tile_pool(name="work", bufs=1) as workp,
            tc.tile_pool(name="outb", bufs=4) as outp,
            tc.tile_pool(name="psum", bufs=8, space="PSUM") as psump,
        ):
            # --------- tiny prep: eps + preload the sqrt/relu ACT table ------
            eps_sb = constp.tile([128, 1], f32, tag="eps")
            nc.vector.memset(eps_sb[:], EPS)
            dummy_sb = workp.tile([128, 1], f32, tag="dummy")
            nc.scalar.activation(dummy_sb[:], eps_sb[:], func=Sqrt, bias=eps_sb[:], scale=1.0)

            # --------- small consts + weights on the scalar HWDGE ring -------
            # (the sync ring is reserved for the big slow/fast input streams)
            wbarT_sb = constp.tile([128, 2, 16], bf16, tag="wbarT")
            for h in range(2):
                nc.scalar.dma_start(wbarT_sb[:, h, :], wbarT_d[h * 128:(h + 1) * 128, :])
            gamma_sb = constp.tile([128, 8], f32, tag="gamma")
            nc.scalar.dma_start(gamma_sb[:], gamma_d[:])
            beta_sb = constp.tile([128, 8], f32, tag="beta")
            nc.scalar.dma_start(beta_sb[:], beta_d[:])
            ident_sb = constp.tile([128, 128], bf16, tag="ident")
            nc.scalar.dma_start(ident_sb[:], ident_d[:])

            posT_sb = constp.tile([128, 2, 2048], bf16, tag="posT")
            for h in range(2):
                nc.scalar.dma_start(posT_sb[:, h, :], posT_d[h * 128:(h + 1) * 128, :])
            convT_sb = constp.tile([128, 16, 1024], bf16, tag="convT")
            convTv = convT_d[:].rearrange("(c p) o -> p c o", p=128)

            # --------- input streams on the sync ring, consumption order -----
            fraw = [
                frawp.tile([128, 4096], bf16, tag="fraw", name=f"fraw{i}")
                for i in range(4)
            ]  # piece (h, t) at index 2*h + t
            sraw = [
                srawp.tile([128, 2, 2048], bf16, tag="sraw", name=f"sraw{i}")
                for i in range(3)
            ]
            sraw67 = [
                srawp.tile([128, 2048], bf16, tag="sraw67", name=f"sraw67_{i}", bufs=2)
                for i in range(2)
            ]
            slowv = slow_d[:].rearrange("(u y p) c -> u p y c", u=4, y=2)

            def dma_fast(h, t):
                nc.sync.dma_start(
                    fraw[2 * h + t][:],
                    fast_d[h * 128:(h + 1) * 128, t * 4096:(t + 1) * 4096],
                )

            nc.sync.dma_start(convT_sb[:, 0:4, :], convTv[:, 0:4, :])
            nc.sync.dma_start(sraw[0][:], slowv[0])
            dma_fast(0, 0)
            dma_fast(0, 1)
            nc.sync.dma_start(convT_sb[:, 8:12, :], convTv[:, 8:12, :])
            nc.sync.dma_start(sraw[1][:], slowv[1])
            dma_fast(1, 0)
            nc.sync.dma_start(convT_sb[:, 12:16, :], convTv[:, 12:16, :])
            dma_fast(1, 1)
            nc.sync.dma_start(convT_sb[:, 4:8, :], convTv[:, 4:8, :])
            nc.sync.dma_start(sraw[2][:], slowv[2])
            nc.sync.dma_start(sraw67[0][:], slow_d[768:896, :])
            nc.sync.dma_start(sraw67[1][:], slow_d[896:1024, :])

            # --------- temporal pooling (sum of 4; x0.25 folded in convT) ----
            # pooled chunk layout: 0..7 slow, 8+2r+h fast
            pooled_rhs = [None] * 16
            pf = [
                poolFp.tile([128, 2, 4, 256], bf16, tag="pf", name=f"pf{i}")
                for i in range(2)
            ]

            def do_fast(h, t):
                v = fraw[2 * h + t][:].rearrange(
                    "p (a b r w) -> p a b r w", a=2, b=2, r=4, w=256
                )
                tmpf = workp.tile([128, 2, 4, 256], bf16, tag="ftmp", bufs=2)
                nc.vector.tensor_add(tmpf[:], v[:, 0], v[:, 1])
                nc.vector.tensor_add(pf[h][:, t], tmpf[:, 0], tmpf[:, 1])

            def do_slow(k):
                s_ap = sraw67[k - 6][:] if k >= 6 else sraw[k // 2][:, k % 2, :]
                v = s_ap.rearrange(
                    "p (t a b w) -> p t a b w", t=2, a=2, b=2, w=256
                )
                eng = nc.gpsimd if k in (2, 3, 4) else nc.vector
                tmp = workp.tile([128, 2, 2, 256], bf16, tag="stmp", bufs=3)
                eng.tensor_add(tmp[:], v[:, :, 0], v[:, :, 1])
                pk = poolSp.tile([128, 2, 256], bf16, tag="ps_slow")
                eng.tensor_add(pk[:], tmp[:, :, 0], tmp[:, :, 1])
                pooled_rhs[k] = pk

            do_slow(0)
            do_slow(1)
            do_fast(0, 0)
            do_fast(0, 1)
            do_slow(2)
            do_slow(3)
            do_fast(1, 0)
            do_fast(1, 1)
            for k in range(4, 8):
                do_slow(k)
            for r in range(4):
                for h in range(2):
                    pooled_rhs[8 + 2 * r + h] = ("fast", h, r)

            # --------- roi_pos fills the PE while inputs stream in -----------
            rpos_sb = constp.tile([16, 2048], f32, tag="rpos")
            # 4-way column-tiled: the four output slices run concurrently in
            # different 32-column groups of the PE array
            rp_all = psump.tile([128, 512], f32, tag="ps", name="rp_all")

            def rpos_mms(h):
                for n in range(4):
                    nc.tensor.matmul(
                        rp_all[32 * n:32 * n + 16, :],
                        lhsT=wbarT_sb[:, h, :],
                        rhs=posT_sb[:, h, n * 512:(n + 1) * 512],
                        start=(h == 0),
                        stop=(h == 1),
                        tile_position=(0, 32 * n),
                    )

            rpos_mms(0)

            # --------- conv matmuls, K-major, ordered by expected arrival ----
            # two M-waves: wave A (m0-4) is DMA-paced; wave B (m5-7) runs dense
            # afterwards while wave A normalizes -> most of the GN/act tail
            # overlaps wave B's matmuls.
            ypsum = [psump.tile([128, 512], f32, tag="ps", name=f"y{i}") for i in range(8)]
            k_order = [0, 1, 2, 3, 8, 10, 12, 14, 9, 11, 13, 15, 4, 5, 6, 7]

            def rhs_of(k):
                pr = pooled_rhs[k]
                if isinstance(pr, tuple):
                    _, h, r = pr
                    return pf[h][:, :, r, :]
                return pr[:]

            def conv_wave(ms):
                first, last = [], None
                for idx, k in enumerate(k_order):
                    rhs = rhs_of(k)
                    for m in ms:
                        inst = nc.tensor.matmul(
                            ypsum[m][:],
                            lhsT=convT_sb[:, k, m * 128:(m + 1) * 128],
                            rhs=rhs,
                            start=(idx == 0),
                            stop=(idx == 15),
                        )
                        if idx == 0:
                            first.append(inst)
                        last = inst
                return first, last

            WAVE_A = [0, 1, 2, 3, 4]
            WAVE_B1 = [5]
            WAVE_B2 = [6, 7]
            _, waveA_last = conv_wave(WAVE_A)
            rpos_mms(1)
            for n in range(4):
                nc.scalar.copy(rpos_sb[:, n * 512:(n + 1) * 512], rp_all[32 * n:32 * n + 16, :])
            nc.scalar.dma_start(rpos_d[:], rpos_sb[:])

            # --------- GroupNorm + ReLU + outputs, one round per wave --------
            # (groups of channel 128m+p are 2m + p//64: each 128-channel tile
            # is self-contained, so wave A normalizes while wave B matmuls)
            featv = feat_d[:].rearrange("(m p t) w -> m p (t w)", m=8, p=128, t=2)
            # featT2 physical layout: [q, h, k, (t, p)] so each transposed pair
            # lands with one contiguous copy; the roi matmul re-orders via AP.
            featT_sb = constp.tile([128, 2, 8, 256], bf16, tag="featT")
            rfeat_sb = constp.tile([16, 2048], f32, tag="rfeat")

            stats3 = workp.tile([128, 8, 3], f32, tag="stats3")
            nmu = workp.tile([128, 8], f32, tag="nmu")
            e2s = workp.tile([128, 8], f32, tag="e2s")
            musq = workp.tile([128, 8], f32, tag="musq")
            var = workp.tile([128, 8], f32, tag="var")
            std = workp.tile([128, 8], f32, tag="std")
            rstd = workp.tile([128, 8], f32, tag="rstd")
            scl = workp.tile([128, 8], f32, tag="scl")
            bias = workp.tile([128, 8], f32, tag="bias")

            def gn_stats(ms):
                for m in ms:
                    bnst = workp.tile([128, 6], f32, tag="bnst", bufs=2)
                    nc.vector.bn_stats(bnst[:], ypsum[m][:])
                    nc.vector.bn_aggr(stats3[:, m, 0:2], bnst[:])
                    nc.vector.tensor_mul(
                        stats3[:, m, 2:3], stats3[:, m, 0:1], stats3[:, m, 0:1]
                    )

            def gn_finish_from(ms, bcs):
                lo, hi = ms[0], ms[-1] + 1
                nm = hi - lo
                sl = slice(lo, hi)
                b3 = bcs[:, 0:3 * nm].rearrange("p (m s) -> p m s", m=nm)
                nc.vector.tensor_scalar_mul(nmu[:, sl], b3[:, :, 0], -1.0 / 64.0)
                nc.vector.tensor_add(e2s[:, sl], b3[:, :, 1], b3[:, :, 2])
                nc.vector.tensor_mul(musq[:, sl], nmu[:, sl], nmu[:, sl])
                nc.vector.tensor_scalar_mul(var[:, sl], e2s[:, sl], 1.0 / 64.0)
                nc.vector.tensor_sub(var[:, sl], var[:, sl], musq[:, sl])
                nc.scalar.activation(std[:, sl], var[:, sl], func=Sqrt, bias=eps_sb[:], scale=1.0)
                nc.vector.reciprocal(rstd[:, sl], std[:, sl])
                nc.vector.tensor_mul(scl[:, sl], gamma_sb[:, sl], rstd[:, sl])
                nc.vector.tensor_mul(bias[:, sl], nmu[:, sl], scl[:, sl])
                nc.vector.tensor_add(bias[:, sl], bias[:, sl], beta_sb[:, sl])

            def act_one(m, on_vector):
                featB = outp.tile([128, 512], bf16, tag="featB", bufs=6)
                if on_vector:
                    nc.vector.tensor_scalar(
                        featB[:], ypsum[m][:], scalar1=scl[:, m:m + 1],
                        scalar2=bias[:, m:m + 1],
                        op0=mybir.AluOpType.mult, op1=mybir.AluOpType.add,
                    )
                    nc.vector.tensor_scalar_max(featB[:], featB[:], 0.0)
                else:
                    nc.scalar.activation(
                        featB[:], ypsum[m][:], func=Relu,
                        bias=bias[:, m:m + 1], scale=scl[:, m:m + 1],
                    )
                # fp32 feat goes out via SWDGE cast-DMA (gpsimd is idle here)
                nc.gpsimd.dma_start(featv[m], featB[:])
                return featB

            def transpose_one(m, featB, copy_vec):
                fBv = featB[:].rearrange("p (t h w) -> p t h w", t=2, h=2, w=128)
                for h in range(2):
                    tp = psump.tile([128, 256], bf16, tag="ps", name=f"tp{m}_{h}")
                    for t in range(2):
                        nc.tensor.matmul(
                            tp[:, t * 128:(t + 1) * 128],
                            lhsT=fBv[:, t, h, :],
                            rhs=ident_sb[:],
                            is_transpose=True,
                            start=True,
                            stop=True,
                        )
                    if (h + (1 if copy_vec else 0)) % 2 == 0:
                        nc.scalar.copy(featT_sb[:, h, m, :], tp[:])
                    else:
                        nc.vector.tensor_copy(featT_sb[:, h, m, :], tp[:])

            def rfeat_pair(n0, cnt=2):
                # roi_feat slices packed into different 32-column groups
                rf = psump.tile([128, 512], f32, tag="ps", name=f"rf{n0}")
                for h in range(2):
                    for j in range(cnt):
                        n = n0 + j
                        rhs = featT_sb[:, h, 2 * n:2 * n + 2, :].rearrange(
                            "q k (t p) -> q k p t", t=2, p=128
                        )
                        nc.tensor.matmul(
                            rf[32 * j:32 * j + 16, :],
                            lhsT=wbarT_sb[:, h, :], rhs=rhs,
                            start=(h == 0), stop=(h == 1),
                            tile_position=(0, 32 * j),
                        )
                for j in range(cnt):
                    n = n0 + j
                    if j == 0:
                        nc.scalar.copy(rfeat_sb[:, n * 512:(n + 1) * 512], rf[32 * j:32 * j + 16, :])
                    else:
                        nc.vector.tensor_copy(rfeat_sb[:, n * 512:(n + 1) * 512], rf[32 * j:32 * j + 16, :])

            # wave A stats; cross-partition sums go through gpsimd so the PE
            # can start wave B with zero stall.  m5 joins GN round A: its stats
            # land right after wave B1 closes, so only m6/m7 remain in the
            # final round.
            gn_stats(WAVE_A)
            waveB1_first, waveB1_last = conv_wave(WAVE_B1)
            for inst in waveB1_first:
                tile.add_dep_helper(inst.ins, waveA_last.ins, reason="wave order")
            gn_stats(WAVE_B1)
            phi = workp.tile([128, 2, 18], f32, tag="phi")
            nc.vector.memset(phi[:], 0.0)
            nc.vector.tensor_copy(
                phi[0:64, 0, :], stats3[0:64, 0:6, :].rearrange("p m s -> p (m s)")
            )
            nc.vector.tensor_copy(
                phi[64:128, 1, :], stats3[64:128, 0:6, :].rearrange("p m s -> p (m s)")
            )
            phi2 = workp.tile([128, 2, 18], f32, tag="phi2")
            nc.gpsimd.partition_all_reduce(
                phi2[:], phi[:], 128, bass_isa.ReduceOp.add
            )
            waveB2_first, _ = conv_wave(WAVE_B2)
            for inst in waveB2_first:
                tile.add_dep_helper(inst.ins, waveB1_last.ins, reason="wave order")
            bcsA = workp.tile([128, 24], f32, tag="bcs", bufs=2)
            nc.vector.tensor_copy(bcsA[0:64, 0:18], phi2[0:64, 0, :])
            nc.vector.tensor_copy(bcsA[64:128, 0:18], phi2[64:128, 1, :])
            gn_finish_from([0, 1, 2, 3, 4, 5], bcsA)
            featBs = {}
            for m in [0, 1, 2, 3, 4, 5]:
                featBs[m] = act_one(m, on_vector=(m % 2 == 1))

            # round-B stats/PAR first in emission order: the vector engine
            # prioritizes the critical GN chain over round-A transpose copies
            gn_stats(WAVE_B2)
            phiB = workp.tile([128, 2, 6], f32, tag="phiB")
            nc.vector.memset(phiB[:], 0.0)
            nc.vector.tensor_copy(
                phiB[0:64, 0, :], stats3[0:64, 6:8, :].rearrange("p m s -> p (m s)")
            )
            nc.vector.tensor_copy(
                phiB[64:128, 1, :], stats3[64:128, 6:8, :].rearrange("p m s -> p (m s)")
            )
            phiB2 = workp.tile([128, 2, 6], f32, tag="phiB2")
            nc.gpsimd.partition_all_reduce(
                phiB2[:], phiB[:], 128, bass_isa.ReduceOp.add
            )
            bcsB = workp.tile([128, 24], f32, tag="bcs", bufs=2)
            nc.vector.tensor_copy(bcsB[0:64, 0:6], phiB2[0:64, 0, :])
            nc.vector.tensor_copy(bcsB[64:128, 0:6], phiB2[64:128, 1, :])
            gn_finish_from([6, 7], bcsB)

            # PE after wave B2: round-A transposes + roi slices, then m6/m7
            for m in [0, 1, 2, 3, 4, 5]:
                transpose_one(m, featBs[m], copy_vec=(m % 2 == 0))
                if m in (3, 5):
                    rfeat_pair(m - 3 if m == 3 else 2, 2 if m == 3 else 1)
            for m in [6, 7]:
                fB = act_one(m, on_vector=(m == 7))
                transpose_one(m, fB, copy_vec=(m % 2 == 0))
                if m == 7:
                    rfeat_pair(3, 1)
            nc.sync.dma_start(rfeat_d[:], rfeat_sb[:])

    nc.compile()
    return nc


def _get_nc():
    if "nc" not in _NC_CACHE:
        _NC_CACHE["nc"] = _build_nc()
    return _NC_CACHE["nc"]


def _roialign_wbar(rois):
    """Mean (over the 16x16 output pixels) RoIAlign weight vector per RoI.

    Numpy port of the reference's roialign_weights followed by mean over P.
    Returns [N, 256] float32.
    """
    rois = np.asarray(rois, np.float32)
    n = rois.shape[0]
    x1 = rois[:, 1] * SCALE - 0.5
    y1 = rois[:, 2] * SCALE - 0.5
    x2 = rois[:, 3] * SCALE - 0.5
    y2 = rois[:, 4] * SCALE - 0.5
    bw = (x2 - x1) / OUT_SIZE
    bh = (y2 - y1) / OUT_SIZE
    grid = (
        np.arange(OUT_SIZE, dtype=np.float32)[:, None]
        + (np.arange(RATIO, dtype=np.float32)[None, :] + 0.5) / RATIO
    )  # [O, r]
    ys = y1[:, None, None] + grid[None] * bh[:, None, None]  # [N, O, r]
    xs = x1[:, None, None] + grid[None] * bw[:, None, None]
    Y = np.broadcast_to(ys[:, :, None, :, None], (n, OUT_SIZE, OUT_SIZE, RATIO, RATIO))
    X = np.broadcast_to(xs[:, None, :, None, :], (n, OUT_SIZE, OUT_SIZE, RATIO, RATIO))
    valid = ((Y >= -1.0) & (Y <= HF) & (X >= -1.0) & (X <= WF)).astype(np.float32)
    y = np.maximum(Y, 0.0)
    x = np.maximum(X, 0.0)
    y0f = np.floor(y)
    x0f = np.floor(x)
    ye = y0f >= HF - 1
    xe = x0f >= WF - 1
    y0 = np.where(ye, HF - 1, y0f).astype(np.int32)
    y1i = np.where(ye, HF - 1, y0f + 1).astype(np.int32)
    x0 = np.where(xe, WF - 1, x0f).astype(np.int32)
    x1i = np.where(xe, WF - 1, x0f + 1).astype(np.int32)
    ly = np.where(ye, 0.0, y - y0f).astype(np.float32)
    lx = np.where(xe, 0.0, x - x0f).astype(np.float32)
    hy = 1.0 - ly
    hx = 1.0 - lx
    cnt = np.float32(RATIO * RATIO)
    w = np.stack([hy * hx, hy * lx, ly * hx, ly * lx], axis=-1) * (valid / cnt)[..., None]
    idx = np.stack(
        [y0 * WF + x0, y0 * WF + x1i, y1i * WF + x0, y1i * WF + x1i], axis=-1
    )
    wbar = np.zeros((n, HF * WF), np.float32)
    flat_idx = idx.reshape(n, -1)
    flat_w = (w / np.float32(OUT_SIZE * OUT_SIZE)).reshape(n, -1)
    np.add.at(wbar, (np.arange(n)[:, None], flat_idx), flat_w)
    return wbar


def _prep_in_maps(slow_feat, fast_feat, rois, pos, conv_w, gn_gamma, gn_beta):
    slow_feat = np.asarray(slow_feat, np.float32)
    fast_feat = np.asarray(fast_feat, np.float32)
    pos = np.asarray(pos, np.float32)
    conv_w = np.asarray(conv_w, np.float32)

    # 1/4 of the temporal mean folded into the conv weight
    convT = np.ascontiguousarray(conv_w.T * np.float32(0.25)).astype(BF16)
    posT = np.ascontiguousarray(pos.reshape(C_FEAT, P_PIX).T).astype(BF16)
    gamma8 = np.ascontiguousarray(
        np.asarray(gn_gamma, np.float32).reshape(8, 128).T
    )
    beta8 = np.ascontiguousarray(np.asarray(gn_beta, np.float32).reshape(8, 128).T)
    ident = np.eye(128, dtype=np.float32).astype(BF16)

    wbar = _roialign_wbar(rois)  # [128, 256]
    # rois are bucketed: N_PER per clip, sorted by batch index (static reshape
    # exactly as in the reference)
    wbarT_all = np.ascontiguousarray(
        wbar.reshape(B, N_PER, P_PIX).transpose(0, 2, 1)
    ).astype(BF16)  # [B, 256, 16]

    in_maps = []
    for b in range(B):
        in_maps.append(
            dict(
                slow=slow_feat[b].reshape(C_SLOW, T_SLOW * P_PIX).astype(BF16),
                fast=fast_feat[b].reshape(C_FAST, T_FAST * P_PIX).astype(BF16),
                convT=convT,
                posT=posT,
                wbarT=np.ascontiguousarray(wbarT_all[b]),
                gamma8=gamma8,
                beta8=beta8,
                ident=ident,
            )
        )
    return in_maps


def _ensure_ntff_hook():
    """Register the axon NTFF profile hook that the boot path skips when the
    image's antenv stub lacks axon_hooks. Test/profiling only."""
    try:
        from antenv.axon_hooks import get_axon_ntff_profile_hook  # noqa: F401
        return
    except ImportError:
        pass
    import types
    import antenv

    if "/root/.axon_site" not in sys.path:
        sys.path.insert(0, "/root/.axon_site")
    from trn_agent_boot.trn_boot import _ntff_profile_via_ctypes

    hook = _ntff_profile_via_ctypes("/opt/axon/libaxon_pjrt.so")
    mod = types.ModuleType("antenv.axon_hooks")
    mod.get_axon_ntff_profile_hook = lambda: hook
    mod.set_axon_ntff_profile_hook = lambda h: None
    sys.modules["antenv.axon_hooks"] = mod
    antenv.axon_hooks = mod

    # artifact upload has no bucket in this container; neuter it
    from concourse import bass_utils

    bass_utils.upload_artifacts = lambda tmpdir: tmpdir


def _run(in_maps, trace=False):
    from concourse.bass_utils import run_bass_kernel_spmd

    if trace:
        _ensure_ntff_hook()
    nc = _get_nc()
    res = run_bass_kernel_spmd(nc, in_maps, core_ids=list(range(B)), trace=trace)
    _NC_CACHE["last_res"] = res
    return res


def _assemble(res):
    feat = np.stack(
        [res.results[b]["feat"].reshape(C_FEAT, HF, WF) for b in range(B)]
    )
    roi_feat = np.stack([res.results[b]["roi_feat"] for b in range(B)])
    roi_pos = np.stack([res.results[b]["roi_pos"] for b in range(B)])
    return feat, roi_feat, roi_pos


def kernel(slow_feat, fast_feat, rois, pos, conv_w, gn_gamma, gn_beta):
    in_maps = _prep_in_maps(slow_feat, fast_feat, rois, pos, conv_w, gn_gamma, gn_beta)
    res = _run(in_maps, trace=False)
    return _assemble(res)


def kernel_traced(slow_feat, fast_feat, rois, pos, conv_w, gn_gamma, gn_beta):
    """Same as kernel() but captures a neuron-profile trace; returns
    (outputs, exec_time_ns)."""
    in_maps = _prep_in_maps(slow_feat, fast_feat, rois, pos, conv_w, gn_gamma, gn_beta)
    res = _run(in_maps, trace=True)
    return _assemble(res), res.exec_time_ns


# revision 28
# speedup vs baseline: 1.1050x; 1.0886x over previous
"""AMCRNet RoI extractor as a Trainium2 Bass/Tile kernel, data-parallel over
the 8 clips (one clip per NeuronCore).

Math notes (derived from the reference):
  - trans_feat = concat(slow, rearranged fast) -> temporal avg-pool (4->1)
    -> 1x1x1 conv (2048->1024) -> GroupNorm(16) -> ReLU -> (c t) fold.
    The avg-pool's 1/4 is folded into the conv weight host-side, so the
    device only does sums of 4 temporal slices.
  - RoIAlign output is immediately averaged over all 256 output pixels, so
    the sparse RoIAlign map collapses to one weight vector per RoI:
      wbar[n, q] = mean_p Wmat[n, p, q]   (q indexes the 16x16 feature map)
    roi_feat[n, c] = sum_q feat[c, q] * wbar[n, q]  -> small matmuls.
  - rois are bucketed per clip (16 per clip, sorted), so wbar rows shard
    alongside the feature maps.
"""

import sys

sys.path.insert(0, "/opt/trn_rl_repo")

import numpy as np
import ml_dtypes

BF16 = ml_dtypes.bfloat16

# problem constants (hardcoded per spec)
B = 8
N_PER = 16
C_SLOW, T_SLOW = 1024, 8
C_FAST, T_FAST = 256, 32
HF, WF = 16, 16
P_PIX = HF * WF  # 256
C_CAT = 2048
C_OUT = 1024
T_POOL = 2
C_FEAT = C_OUT * T_POOL  # 2048
OUT_SIZE = 16
RATIO = 2
SCALE = 1.0 / 16.0
GN_GROUPS = 16
EPS = 1e-5

_NC_CACHE = {}


def _build_nc():
    import concourse.bacc as bacc
    import concourse.tile as tile
    from concourse import mybir
    from concourse import bass_isa

    f32 = mybir.dt.float32
    bf16 = mybir.dt.bfloat16
    Relu = mybir.ActivationFunctionType.Relu
    Sqrt = mybir.ActivationFunctionType.Sqrt

    nc = bacc.Bacc("TRN2", target_bir_lowering=False, debug=False)

    # ---- per-core DRAM parameters (slow/fast pre-cast to bf16 on host) ----
    slow_d = nc.dram_tensor("slow", [1024, 2048], bf16, kind="ExternalInput")
    fast_d = nc.dram_tensor("fast", [256, 8192], bf16, kind="ExternalInput")
    convT_d = nc.dram_tensor("convT", [2048, 1024], bf16, kind="ExternalInput")
    posT_d = nc.dram_tensor("posT", [256, 2048], bf16, kind="ExternalInput")
    wbarT_d = nc.dram_tensor("wbarT", [256, 16], bf16, kind="ExternalInput")
    gamma_d = nc.dram_tensor("gamma8", [128, 8], f32, kind="ExternalInput")
    beta_d = nc.dram_tensor("beta8", [128, 8], f32, kind="ExternalInput")
    ident_d = nc.dram_tensor("ident", [128, 128], bf16, kind="ExternalInput")

    feat_d = nc.dram_tensor("feat", [2048, 256], f32, kind="ExternalOutput")
    rfeat_d = nc.dram_tensor("roi_feat", [16, 2048], f32, kind="ExternalOutput")
    rpos_d = nc.dram_tensor("roi_pos", [16, 2048], f32, kind="ExternalOutput")

    with tile.TileContext(nc) as tc:
        with (
            tc.tile_pool(name="const", bufs=1) as constp,
            tc.tile_pool(name="sraw", bufs=4) as srawp,
            tc.tile_pool(name="fraw", bufs=4) as frawp,
            tc.tile_pool(name="pooledS", bufs=8) as poolSp,
            tc.tile_pool(name="pooledF", bufs=2) as poolFp,
            tc.tile_pool(name="work", bufs=1) as workp,
            tc.tile_pool(name="outb", bufs=4) as outp,
            tc.tile_pool(name="psum", bufs=8, space="PSUM") as psump,
        ):
            # --------- tiny prep: eps + preload the sqrt/relu ACT table ------
            eps_sb = constp.tile([128, 1], f32, tag="eps")
            nc.vector.memset(eps_sb[:], EPS)
            dummy_sb = workp.tile([128, 1], f32, tag="dummy")
            nc.scalar.activation(dummy_sb[:], eps_sb[:], func=Sqrt, bias=eps_sb[:], scale=1.0)

            # --------- small consts + weights on the scalar HWDGE ring -------
            # (the sync ring is reserved for the big slow/fast input streams)
            wbarT_sb = constp.tile([128, 2, 16], bf16, tag="wbarT")
            for h in range(2):
                nc.scalar.dma_start(wbarT_sb[:, h, :], wbarT_d[h * 128:(h + 1) * 128, :])
            gamma_sb = constp.tile([128, 8], f32, tag="gamma")
            nc.scalar.dma_start(gamma_sb[:], gamma_d[:])
            beta_sb = constp.tile([128, 8], f32, tag="beta")
            nc.scalar.dma_start(beta_sb[:], beta_d[:])
            ident_sb = constp.tile([128, 128], bf16, tag="ident")
            nc.scalar.dma_start(ident_sb[:], ident_d[:])

            posT_sb = constp.tile([128, 2, 2048], bf16, tag="posT")
            for h in range(2):
                nc.scalar.dma_start(posT_sb[:, h, :], posT_d[h * 128:(h + 1) * 128, :])
            convT_sb = constp.tile([128, 16, 1024], bf16, tag="convT")
            convTv = convT_d[:].rearrange("(c p) o -> p c o", p=128)

            # --------- input streams on the sync ring, consumption order -----
            fraw = [
                frawp.tile([128, 4096], bf16, tag="fraw", name=f"fraw{i}")
                for i in range(4)
            ]  # piece (h, t) at index 2*h + t
            sraw = [
                srawp.tile([128, 2, 2048], bf16, tag="sraw", name=f"sraw{i}")
                for i in range(3)
            ]
            sraw67 = [
                srawp.tile([128, 2048], bf16, tag="sraw67", name=f"sraw67_{i}", bufs=2)
                for i in range(2)
            ]
            slowv = slow_d[:].rearrange("(u y p) c -> u p y c", u=4, y=2)

            def dma_fast(h, t):
                nc.sync.dma_start(
                    fraw[2 * h + t][:],
                    fast_d[h * 128:(h + 1) * 128, t * 4096:(t + 1) * 4096],
                )

            nc.sync.dma_start(convT_sb[:, 0:4, :], convTv[:, 0:4, :])
            nc.sync.dma_start(sraw[0][:], slowv[0])
            dma_fast(0, 0)
            dma_fast(0, 1)
            nc.sync.dma_start(convT_sb[:, 8:12, :], convTv[:, 8:12, :])
            nc.sync.dma_start(sraw[1][:], slowv[1])
            dma_fast(1, 0)
            nc.sync.dma_start(convT_sb[:, 12:16, :], convTv[:, 12:16, :])
            dma_fast(1, 1)
            nc.sync.dma_start(convT_sb[:, 4:8, :], convTv[:, 4:8, :])
            nc.sync.dma_start(sraw[2][:], slowv[2])
            nc.sync.dma_start(sraw67[0][:], slow_d[768:896, :])
            nc.sync.dma_start(sraw67[1][:], slow_d[896:1024, :])

            # --------- temporal pooling (sum of 4; x0.25 folded in convT) ----
            # pooled chunk layout: 0..7 slow, 8+2r+h fast
            pooled_rhs = [None] * 16
            pf = [
                poolFp.tile([128, 2, 4, 256], bf16, tag="pf", name=f"pf{i}")
                for i in range(2)
            ]

            def do_fast(h, t):
                v = fraw[2 * h + t][:].rearrange(
                    "p (a b r w) -> p a b r w", a=2, b=2, r=4, w=256
                )
                tmpf = workp.tile([128, 2, 4, 256], bf16, tag="ftmp", bufs=2)
                nc.vector.tensor_add(tmpf[:], v[:, 0], v[:, 1])
                nc.vector.tensor_add(pf[h][:, t], tmpf[:, 0], tmpf[:, 1])

            def do_slow(k):
                s_ap = sraw67[k - 6][:] if k >= 6 else sraw[k // 2][:, k % 2, :]
                v = s_ap.rearrange(
                    "p (t a b w) -> p t a b w", t=2, a=2, b=2, w=256
                )
                eng = nc.gpsimd if k in (2, 3, 4) else nc.vector
                tmp = workp.tile([128, 2, 2, 256], bf16, tag="stmp", bufs=3)
                eng.tensor_add(tmp[:], v[:, :, 0], v[:, :, 1])
                pk = poolSp.tile([128, 2, 256], bf16, tag="ps_slow")
                eng.tensor_add(pk[:], tmp[:, :, 0], tmp[:, :, 1])
                pooled_rhs[k] = pk

            do_slow(0)
            do_slow(1)
            do_fast(0, 0)
            do_fast(0, 1)
            do_slow(2)
            do_slow(3)
            do_fast(1, 0)
            do_fast(1, 1)
            for k in range(4, 8):
                do_slow(k)
            for r in range(4):
                for h in range(2):
                    pooled_rhs[8 + 2 * r + h] = ("fast", h, r)

            # --------- roi_pos fills the PE while inputs stream in -----------
            rpos_sb = constp.tile([16, 2048], f32, tag="rpos")
            # 4-way column-tiled: the four output slices run concurrently in
            # different 32-column groups of the PE array
            rp_all = psump.tile([128, 512], f32, tag="ps", name="rp_all")

            def rpos_mms(h):
                for n in range(4):
                    nc.tensor.matmul(
                        rp_all[32 * n:32 * n + 16, :],
                        lhsT=wbarT_sb[:, h, :],
                        rhs=posT_sb[:, h, n * 512:(n + 1) * 512],
                        start=(h == 0),
                        stop=(h == 1),
                        tile_position=(0, 32 * n),
                    )

            rpos_mms(0)

            # --------- conv matmuls, K-major, ordered by expected arrival ----
            # two M-waves: wave A (m0-4) is DMA-paced; wave B (m5-7) runs dense
            # afterwards while wave A normalizes -> most of the GN/act tail
            # overlaps wave B's matmuls.
            ypsum = [psump.tile([128, 512], f32, tag="ps", name=f"y{i}") for i in range(8)]
            k_order = [0, 1, 2, 3, 8, 10, 12, 14, 9, 11, 13, 15, 4, 5, 6, 7]

            def rhs_of(k):
                pr = pooled_rhs[k]
                if isinstance(pr, tuple):
                    _, h, r = pr
                    return pf[h][:, :, r, :]
                return pr[:]

            def conv_wave(ms):
                first, last = [], None
                for idx, k in enumerate(k_order):
                    rhs = rhs_of(k)
                    for m in ms:
                        inst = nc.tensor.matmul(
                            ypsum[m][:],
                            lhsT=convT_sb[:, k, m * 128:(m + 1) * 128],
                            rhs=rhs,
                            start=(idx == 0),
                            stop=(idx == 15),
                        )
                        if idx == 0:
                            first.append(inst)
                        last = inst
                return first, last

            WAVE_A = [0, 1, 2, 3, 4]
            WAVE_B1 = [5]
            WAVE_B2 = [6, 7]
            _, waveA_last = conv_wave(WAVE_A)
            rpos_mms(1)
            for n in range(4):
                nc.scalar.copy(rpos_sb[:, n * 512:(n + 1) * 512], rp_all[32 * n:32 * n + 16, :])
            nc.scalar.dma_start(rpos_d[:], rpos_sb[:])

            # --------- GroupNorm + ReLU + outputs, one round per wave --------
            # (groups of channel 128m+p are 2m + p//64: each 128-channel tile
            # is self-contained, so wave A normalizes while wave B matmuls)
            featv = feat_d[:].rearrange("(m p t) w -> m p (t w)", m=8, p=128, t=2)
            # featT2 physical layout: [q, h, k, (t, p)] so each transposed pair
            # lands with one contiguous copy; the roi matmul re-orders via AP.
            featT_sb = constp.tile([128, 2, 8, 256], bf16, tag="featT")
            rfeat_sb = constp.tile([16, 2048], f32, tag="rfeat")

            stats3 = workp.tile([128, 8, 3], f32, tag="stats3")
            nmu = workp.tile([128, 8], f32, tag="nmu")
            e2s = workp.tile([128, 8], f32, tag="e2s")
            musq = workp.tile([128, 8], f32, tag="musq")
            var = workp.tile([128, 8], f32, tag="var")
            std = workp.tile([128, 8], f32, tag="std")
            rstd = workp.tile([128, 8], f32, tag="rstd")
            scl = workp.tile([128, 8], f32, tag="scl")
            bias = workp.tile([128, 8], f32, tag="bias")

            def gn_stats(ms):
                for m in ms:
                    bnst = workp.tile([128, 6], f32, tag="bnst", bufs=2)
                    nc.vector.bn_stats(bnst[:], ypsum[m][:])
                    nc.vector.bn_aggr(stats3[:, m, 0:2], bnst[:])
                    nc.vector.tensor_mul(
                        stats3[:, m, 2:3], stats3[:, m, 0:1], stats3[:, m, 0:1]
                    )

            def gn_finish_from(ms, bcs):
                lo, hi = ms[0], ms[-1] + 1
                nm = hi - lo
                sl = slice(lo, hi)
                b3 = bcs[:, 0:3 * nm].rearrange("p (m s) -> p m s", m=nm)
                nc.vector.tensor_scalar_mul(nmu[:, sl], b3[:, :, 0], -1.0 / 64.0)
                nc.vector.tensor_add(e2s[:, sl], b3[:, :, 1], b3[:, :, 2])
                nc.vector.tensor_mul(musq[:, sl], nmu[:, sl], nmu[:, sl])
                nc.vector.tensor_scalar_mul(var[:, sl], e2s[:, sl], 1.0 / 64.0)
                nc.vector.tensor_sub(var[:, sl], var[:, sl], musq[:, sl])
                nc.scalar.activation(std[:, sl], var[:, sl], func=Sqrt, bias=eps_sb[:], scale=1.0)
                nc.vector.reciprocal(rstd[:, sl], std[:, sl])
                nc.vector.tensor_mul(scl[:, sl], gamma_sb[:, sl], rstd[:, sl])
                nc.vector.tensor_mul(bias[:, sl], nmu[:, sl], scl[:, sl])
                nc.vector.tensor_add(bias[:, sl], bias[:, sl], beta_sb[:, sl])

            def act_one(m, on_vector):
                featB = outp.tile([128, 512], bf16, tag="featB", bufs=6)
                if on_vector:
                    nc.vector.tensor_scalar(
                        featB[:], ypsum[m][:], scalar1=scl[:, m:m + 1],
                        scalar2=bias[:, m:m + 1],
                        op0=mybir.AluOpType.mult, op1=mybir.AluOpType.add,
                    )
                    nc.vector.tensor_scalar_max(featB[:], featB[:], 0.0)
                else:
                    nc.scalar.activation(
                        featB[:], ypsum[m][:], func=Relu,
                        bias=bias[:, m:m + 1], scale=scl[:, m:m + 1],
                    )
                # fp32 feat goes out via SWDGE cast-DMA (gpsimd is idle here)
                nc.gpsimd.dma_start(featv[m], featB[:])
                return featB

            def transpose_one(m, featB, copy_vec):
                fBv = featB[:].rearrange("p (t h w) -> p t h w", t=2, h=2, w=128)
                for h in range(2):
                    tp = psump.tile([128, 256], bf16, tag="ps", name=f"tp{m}_{h}")
                    for t in range(2):
                        nc.tensor.matmul(
                            tp[:, t * 128:(t + 1) * 128],
                            lhsT=fBv[:, t, h, :],
                            rhs=ident_sb[:],
                            is_transpose=True,
                            start=True,
                            stop=True,
                        )
                    if (h + (1 if copy_vec else 0)) % 2 == 0:
                        nc.scalar.copy(featT_sb[:, h, m, :], tp[:])
                    else:
                        nc.vector.tensor_copy(featT_sb[:, h, m, :], tp[:])

            def rfeat_pair(n0, cnt=2):
                # roi_feat slices packed into different 32-column groups
                rf = psump.tile([128, 512], f32, tag="ps", name=f"rf{n0}")
                for h in range(2):
                    for j in range(cnt):
                        n = n0 + j
                        rhs = featT_sb[:, h, 2 * n:2 * n + 2, :].rearrange(
                            "q k (t p) -> q k p t", t=2, p=128
                        )
                        nc.tensor.matmul(
                            rf[32 * j:32 * j + 16, :],
                            lhsT=wbarT_sb[:, h, :], rhs=rhs,
                            start=(h == 0), stop=(h == 1),
                            tile_position=(0, 32 * j),
                        )
                for j in range(cnt):
                    n = n0 + j
                    if j == 0:
                        nc.scalar.copy(rfeat_sb[:, n * 512:(n + 1) * 512], rf[32 * j:32 * j + 16, :])
                    else:
                        nc.vector.tensor_copy(rfeat_sb[:, n * 512:(n + 1) * 512], rf[32 * j:32 * j + 16, :])

            # wave A stats; cross-partition sums go through gpsimd so the PE
            # can start wave B with zero stall.  m5 joins GN round A: its stats
            # land right after wave B1 closes, so only m6/m7 remain in the
            # final round.
            gn_stats(WAVE_A)
            waveB1_first, waveB1_last = conv_wave(WAVE_B1)
            for inst in waveB1_first:
                tile.add_dep_helper(inst.ins, waveA_last.ins, reason="wave order")
            gn_stats(WAVE_B1)
            phi = workp.tile([128, 2, 18], f32, tag="phi")
            nc.vector.memset(phi[:], 0.0)
            nc.vector.tensor_copy(
                phi[0:64, 0, :], stats3[0:64, 0:6, :].rearrange("p m s -> p (m s)")
            )
            nc.vector.tensor_copy(
                phi[64:128, 1, :], stats3[64:128, 0:6, :].rearrange("p m s -> p (m s)")
            )
            phi2 = workp.tile([128, 2, 18], f32, tag="phi2")
            nc.gpsimd.partition_all_reduce(
                phi2[:], phi[:], 128, bass_isa.ReduceOp.add
            )
            waveB2_first, _ = conv_wave(WAVE_B2)
            for inst in waveB2_first:
                tile.add_dep_helper(inst.ins, waveB1_last.ins, reason="wave order")
            bcsA = workp.tile([128, 24], f32, tag="bcs", bufs=2)
            nc.vector.tensor_copy(bcsA[0:64, 0:18], phi2[0:64, 0, :])
            nc.vector.tensor_copy(bcsA[64:128, 0:18], phi2[64:128, 1, :])
            gn_finish_from([0, 1, 2, 3, 4, 5], bcsA)
            featBs = {}
            for m in [0, 1, 2, 3, 4, 5]:
                featBs[m] = act_one(m, on_vector=(m % 2 == 1))

            # round-B stats/PAR first in emission order: the vector engine
            # prioritizes the critical GN chain over round-A transpose copies
            gn_stats(WAVE_B2)
            phiB = workp.tile([128, 2, 6], f32, tag="phiB")
            nc.vector.memset(phiB[:], 0.0)
            nc.vector.tensor_copy(
                phiB[0:64, 0, :], stats3[0:64, 6:8, :].rearrange("p m s -> p (m s)")
            )
            nc.vector.tensor_copy(
                phiB[64:128, 1, :], stats3[64:128, 6:8, :].rearrange("p m s -> p (m s)")
            )
            phiB2 = workp.tile([128, 2, 6], f32, tag="phiB2")
            nc.gpsimd.partition_all_reduce(
                phiB2[:], phiB[:], 128, bass_isa.ReduceOp.add
            )
            bcsB = workp.tile([128, 24], f32, tag="bcs", bufs=2)
            nc.vector.tensor_copy(bcsB[0:64, 0:6], phiB2[0:64, 0, :])
            nc.vector.tensor_copy(bcsB[64:128, 0:6], phiB2[64:128, 1, :])
            gn_finish_from([6, 7], bcsB)

            # PE after wave B2: round-A transposes + roi slices, then m6/m7
            for m in [0, 1, 2, 3, 4, 5]:
                transpose_one(m, featBs[m], copy_vec=(m % 2 == 0))
                if m in (3, 5):
                    rfeat_pair(m - 3 if m == 3 else 2, 2 if m == 3 else 1)
            for m in [6, 7]:
                fB = act_one(m, on_vector=(m == 7))
                transpose_one(m, fB, copy_vec=(m % 2 == 0))
                if m == 7:
                    rfeat_pair(3, 1)
            nc.sync.dma_start(rfeat_d[:], rfeat_sb[:])

    nc.compile()
    return nc


def _get_nc():
    if "nc" not in _NC_CACHE:
        _NC_CACHE["nc"] = _build_nc()
    return _NC_CACHE["nc"]


def _roialign_wbar(rois):
    """Mean (over the 16x16 output pixels) RoIAlign weight vector per RoI.

    Numpy port of the reference's roialign_weights followed by mean over P.
    Returns [N, 256] float32.
    """
    rois = np.asarray(rois, np.float32)
    n = rois.shape[0]
    x1 = rois[:, 1] * SCALE - 0.5
    y1 = rois[:, 2] * SCALE - 0.5
    x2 = rois[:, 3] * SCALE - 0.5
    y2 = rois[:, 4] * SCALE - 0.5
    bw = (x2 - x1) / OUT_SIZE
    bh = (y2 - y1) / OUT_SIZE
    grid = (
        np.arange(OUT_SIZE, dtype=np.float32)[:, None]
        + (np.arange(RATIO, dtype=np.float32)[None, :] + 0.5) / RATIO
    )  # [O, r]
    ys = y1[:, None, None] + grid[None] * bh[:, None, None]  # [N, O, r]
    xs = x1[:, None, None] + grid[None] * bw[:, None, None]
    Y = np.broadcast_to(ys[:, :, None, :, None], (n, OUT_SIZE, OUT_SIZE, RATIO, RATIO))
    X = np.broadcast_to(xs[:, None, :, None, :], (n, OUT_SIZE, OUT_SIZE, RATIO, RATIO))
    valid = ((Y >= -1.0) & (Y <= HF) & (X >= -1.0) & (X <= WF)).astype(np.float32)
    y = np.maximum(Y, 0.0)
    x = np.maximum(X, 0.0)
    y0f = np.floor(y)
    x0f = np.floor(x)
    ye = y0f >= HF - 1
    xe = x0f >= WF - 1
    y0 = np.where(ye, HF - 1, y0f).astype(np.int32)
    y1i = np.where(ye, HF - 1, y0f + 1).astype(np.int32)
    x0 = np.where(xe, WF - 1, x0f).astype(np.int32)
    x1i = np.where(xe, WF - 1, x0f + 1).astype(np.int32)
    ly = np.where(ye, 0.0, y - y0f).astype(np.float32)
    lx = np.where(xe, 0.0, x - x0f).astype(np.float32)
    hy = 1.0 - ly
    hx = 1.0 - lx
    cnt = np.float32(RATIO * RATIO)
    w = np.stack([hy * hx, hy * lx, ly * hx, ly * lx], axis=-1) * (valid / cnt)[..., None]
    idx = np.stack(
        [y0 * WF + x0, y0 * WF + x1i, y1i * WF + x0, y1i * WF + x1i], axis=-1
    )
    wbar = np.zeros((n, HF * WF), np.float32)
    flat_idx = idx.reshape(n, -1)
    flat_w = (w / np.float32(OUT_SIZE * OUT_SIZE)).reshape(n, -1)
    np.add.at(wbar, (np.arange(n)[:, None], flat_idx), flat_w)
    return wbar


def _prep_in_maps(slow_feat, fast_feat, rois, pos, conv_w, gn_gamma, gn_beta):
    slow_feat = np.asarray(slow_feat, np.float32)
    fast_feat = np.asarray(fast_feat, np.float32)
    pos = np.asarray(pos, np.float32)
    conv_w = np.asarray(conv_w, np.float32)

    # 1/4 of the temporal mean folded into the conv weight
    convT = np.ascontiguousarray(conv_w.T * np.float32(0.25)).astype(BF16)
    posT = np.ascontiguousarray(pos.reshape(C_FEAT, P_PIX).T).astype(BF16)
    gamma8 = np.ascontiguousarray(
        np.asarray(gn_gamma, np.float32).reshape(8, 128).T
    )
    beta8 = np.ascontiguousarray(np.asarray(gn_beta, np.float32).reshape(8, 128).T)
    ident = np.eye(128, dtype=np.float32).astype(BF16)

    wbar = _roialign_wbar(rois)  # [128, 256]
    # rois are bucketed: N_PER per clip, sorted by batch index (static reshape
    # exactly as in the reference)
    wbarT_all = np.ascontiguousarray(
        wbar.reshape(B, N_PER, P_PIX).transpose(0, 2, 1)
    ).astype(BF16)  # [B, 256, 16]

    in_maps = []
    for b in range(B):
        in_maps.append(
            dict(
                slow=slow_feat[b].reshape(C_SLOW, T_SLOW * P_PIX).astype(BF16),
                fast=fast_feat[b].reshape(C_FAST, T_FAST * P_PIX).astype(BF16),
                convT=convT,
                posT=posT,
                wbarT=np.ascontiguousarray(wbarT_all[b]),
                gamma8=gamma8,
                beta8=beta8,
                ident=ident,
            )
        )
    return in_maps


def _ensure_ntff_hook():
    """Register the axon NTFF profile hook that the boot path skips when the
    image's antenv stub lacks axon_hooks. Test/profiling only."""
    try:
        from antenv.axon_hooks import get_axon_ntff_profile_hook  # noqa: F401
        return
    except ImportError:
        pass
    import types
    import antenv

    if "/root/.axon_site" not in sys.path:
        sys.path.insert(0, "/root/.axon_site")
    from trn_agent_boot.trn_boot import _ntff_profile_via_ctypes

    hook = _ntff_profile_via_ctypes("/opt/axon/libaxon_pjrt.so")
    mod = types.ModuleType("antenv.axon_hooks")
    mod.get_axon_ntff_profile_hook = lambda: hook
    mod.set_axon_ntff_profile_hook = lambda h: None
    sys.modules["antenv.axon_hooks"] = mod
    antenv.axon_hooks = mod

    # artifact upload has no bucket in this container; neuter it
    from concourse import bass_utils

    bass_utils.upload_artifacts = lambda tmpdir: tmpdir


def _run(in_maps, trace=False):
    from concourse.bass_utils import run_bass_kernel_spmd

    if trace:
        _ensure_ntff_hook()
    nc = _get_nc()
    res = run_bass_kernel_spmd(nc, in_maps, core_ids=list(range(B)), trace=trace)
    _NC_CACHE["last_res"] = res
    return res


def _assemble(res):
    feat = np.stack(
        [res.results[b]["feat"].reshape(C_FEAT, HF, WF) for b in range(B)]
    )
    roi_feat = np.stack([res.results[b]["roi_feat"] for b in range(B)])
    roi_pos = np.stack([res.results[b]["roi_pos"] for b in range(B)])
    return feat, roi_feat, roi_pos


def kernel(slow_feat, fast_feat, rois, pos, conv_w, gn_gamma, gn_beta):
    in_maps = _prep_in_maps(slow_feat, fast_feat, rois, pos, conv_w, gn_gamma, gn_beta)
    res = _run(in_maps, trace=False)
    return _assemble(res)


def kernel_traced(slow_feat, fast_feat, rois, pos, conv_w, gn_gamma, gn_beta):
    """Same as kernel() but captures a neuron-profile trace; returns
    (outputs, exec_time_ns)."""
    in_maps = _prep_in_maps(slow_feat, fast_feat, rois, pos, conv_w, gn_gamma, gn_beta)
    res = _run(in_maps, trace=True)
    return _assemble(res), res.exec_time_ns


# revision 30
# speedup vs baseline: 1.1356x; 1.0276x over previous
"""AMCRNet RoI extractor as a Trainium2 Bass/Tile kernel, data-parallel over
the 8 clips (one clip per NeuronCore).

Math notes (derived from the reference):
  - trans_feat = concat(slow, rearranged fast) -> temporal avg-pool (4->1)
    -> 1x1x1 conv (2048->1024) -> GroupNorm(16) -> ReLU -> (c t) fold.
    The avg-pool's 1/4 is folded into the conv weight host-side, so the
    device only does sums of 4 temporal slices.
  - RoIAlign output is immediately averaged over all 256 output pixels, so
    the sparse RoIAlign map collapses to one weight vector per RoI:
      wbar[n, q] = mean_p Wmat[n, p, q]   (q indexes the 16x16 feature map)
    roi_feat[n, c] = sum_q feat[c, q] * wbar[n, q]  -> small matmuls.
  - rois are bucketed per clip (16 per clip, sorted), so wbar rows shard
    alongside the feature maps.
"""

import sys

sys.path.insert(0, "/opt/trn_rl_repo")

import numpy as np
import ml_dtypes

BF16 = ml_dtypes.bfloat16

# problem constants (hardcoded per spec)
B = 8
N_PER = 16
C_SLOW, T_SLOW = 1024, 8
C_FAST, T_FAST = 256, 32
HF, WF = 16, 16
P_PIX = HF * WF  # 256
C_CAT = 2048
C_OUT = 1024
T_POOL = 2
C_FEAT = C_OUT * T_POOL  # 2048
OUT_SIZE = 16
RATIO = 2
SCALE = 1.0 / 16.0
GN_GROUPS = 16
EPS = 1e-5

_NC_CACHE = {}


def _build_nc():
    import concourse.bacc as bacc
    import concourse.tile as tile
    from concourse import mybir
    from concourse import bass_isa

    f32 = mybir.dt.float32
    bf16 = mybir.dt.bfloat16
    Relu = mybir.ActivationFunctionType.Relu
    Sqrt = mybir.ActivationFunctionType.Sqrt

    nc = bacc.Bacc("TRN2", target_bir_lowering=False, debug=False)

    # ---- per-core DRAM parameters (slow/fast pre-cast to bf16 on host) ----
    slow_d = nc.dram_tensor("slow", [1024, 2048], bf16, kind="ExternalInput")
    fast_d = nc.dram_tensor("fast", [256, 8192], bf16, kind="ExternalInput")
    convT_d = nc.dram_tensor("convT", [2048, 1024], bf16, kind="ExternalInput")
    posT_d = nc.dram_tensor("posT", [256, 2048], bf16, kind="ExternalInput")
    wbarT_d = nc.dram_tensor("wbarT", [256, 16], bf16, kind="ExternalInput")
    gamma_d = nc.dram_tensor("gamma8", [128, 8], f32, kind="ExternalInput")
    beta_d = nc.dram_tensor("beta8", [128, 8], f32, kind="ExternalInput")
    ident_d = nc.dram_tensor("ident", [128, 128], bf16, kind="ExternalInput")

    feat_d = nc.dram_tensor("feat", [2048, 256], f32, kind="ExternalOutput")
    rfeat_d = nc.dram_tensor("roi_feat", [16, 2048], f32, kind="ExternalOutput")
    rpos_d = nc.dram_tensor("roi_pos", [16, 2048], f32, kind="ExternalOutput")

    with tile.TileContext(nc) as tc:
        with (
            tc.tile_pool(name="const", bufs=1) as constp,
            tc.tile_pool(name="sraw", bufs=4) as srawp,
            tc.tile_pool(name="fraw", bufs=4) as frawp,
            tc.tile_pool(name="pooledS", bufs=8) as poolSp,
            tc.tile_pool(name="pooledF", bufs=2) as poolFp,
            tc.tile_pool(name="work", bufs=1) as workp,
            tc.tile_pool(name="outb", bufs=4) as outp,
            tc.tile_pool(name="psum", bufs=8, space="PSUM") as psump,
        ):
            # --------- tiny prep: eps + preload the sqrt/relu ACT table ------
            eps_sb = constp.tile([128, 1], f32, tag="eps")
            nc.vector.memset(eps_sb[:], EPS)
            dummy_sb = workp.tile([128, 1], f32, tag="dummy")
            nc.scalar.activation(dummy_sb[:], eps_sb[:], func=Sqrt, bias=eps_sb[:], scale=1.0)

            # --------- small consts + weights on the scalar HWDGE ring -------
            # (the sync ring is reserved for the big slow/fast input streams)
            wbarT_sb = constp.tile([128, 2, 16], bf16, tag="wbarT")
            for h in range(2):
                nc.scalar.dma_start(wbarT_sb[:, h, :], wbarT_d[h * 128:(h + 1) * 128, :])
            gamma_sb = constp.tile([128, 8], f32, tag="gamma")
            nc.scalar.dma_start(gamma_sb[:], gamma_d[:])
            beta_sb = constp.tile([128, 8], f32, tag="beta")
            nc.scalar.dma_start(beta_sb[:], beta_d[:])
            ident_sb = constp.tile([128, 128], bf16, tag="ident")
            nc.scalar.dma_start(ident_sb[:], ident_d[:])

            posT_sb = constp.tile([128, 2, 2048], bf16, tag="posT")
            for h in range(2):
                nc.scalar.dma_start(posT_sb[:, h, :], posT_d[h * 128:(h + 1) * 128, :])
            convT_sb = constp.tile([128, 16, 1024], bf16, tag="convT")
            convTv = convT_d[:].rearrange("(c p) o -> p c o", p=128)

            # --------- input streams on the sync ring, consumption order -----
            fraw = [
                frawp.tile([128, 4096], bf16, tag="fraw", name=f"fraw{i}")
                for i in range(4)
            ]  # piece (h, t) at index 2*h + t
            sraw = [
                srawp.tile([128, 2, 2048], bf16, tag="sraw", name=f"sraw{i}")
                for i in range(3)
            ]
            sraw67 = [
                srawp.tile([128, 2048], bf16, tag="sraw67", name=f"sraw67_{i}", bufs=2)
                for i in range(2)
            ]
            slowv = slow_d[:].rearrange("(u y p) c -> u p y c", u=4, y=2)

            def dma_fast(h, t):
                nc.sync.dma_start(
                    fraw[2 * h + t][:],
                    fast_d[h * 128:(h + 1) * 128, t * 4096:(t + 1) * 4096],
                )

            nc.sync.dma_start(convT_sb[:, 0:4, :], convTv[:, 0:4, :])
            nc.sync.dma_start(sraw[0][:], slowv[0])
            dma_fast(0, 0)
            dma_fast(0, 1)
            nc.sync.dma_start(convT_sb[:, 8:12, :], convTv[:, 8:12, :])
            nc.sync.dma_start(sraw[1][:], slowv[1])
            dma_fast(1, 0)
            nc.sync.dma_start(convT_sb[:, 12:16, :], convTv[:, 12:16, :])
            dma_fast(1, 1)
            nc.sync.dma_start(convT_sb[:, 4:8, :], convTv[:, 4:8, :])
            nc.sync.dma_start(sraw[2][:], slowv[2])
            nc.sync.dma_start(sraw67[0][:], slow_d[768:896, :])
            nc.sync.dma_start(sraw67[1][:], slow_d[896:1024, :])

            # --------- temporal pooling (sum of 4; x0.25 folded in convT) ----
            # pooled chunk layout: 0..7 slow, 8+2r+h fast
            pooled_rhs = [None] * 16
            pf = [
                poolFp.tile([128, 2, 4, 256], bf16, tag="pf", name=f"pf{i}")
                for i in range(2)
            ]

            def do_fast(h, t):
                v = fraw[2 * h + t][:].rearrange(
                    "p (a b r w) -> p a b r w", a=2, b=2, r=4, w=256
                )
                tmpf = workp.tile([128, 2, 4, 256], bf16, tag="ftmp", bufs=2)
                nc.vector.tensor_add(tmpf[:], v[:, 0], v[:, 1])
                nc.vector.tensor_add(pf[h][:, t], tmpf[:, 0], tmpf[:, 1])

            def do_slow(k):
                s_ap = sraw67[k - 6][:] if k >= 6 else sraw[k // 2][:, k % 2, :]
                v = s_ap.rearrange(
                    "p (t a b w) -> p t a b w", t=2, a=2, b=2, w=256
                )
                eng = nc.gpsimd if k in (2, 3, 4) else nc.vector
                tmp = workp.tile([128, 2, 2, 256], bf16, tag="stmp", bufs=3)
                eng.tensor_add(tmp[:], v[:, :, 0], v[:, :, 1])
                pk = poolSp.tile([128, 2, 256], bf16, tag="ps_slow")
                eng.tensor_add(pk[:], tmp[:, :, 0], tmp[:, :, 1])
                pooled_rhs[k] = pk

            do_slow(0)
            do_slow(1)
            do_fast(0, 0)
            do_fast(0, 1)
            do_slow(2)
            do_slow(3)
            do_fast(1, 0)
            do_fast(1, 1)
            for k in range(4, 8):
                do_slow(k)
            for r in range(4):
                for h in range(2):
                    pooled_rhs[8 + 2 * r + h] = ("fast", h, r)

            # --------- roi_pos fills the PE while inputs stream in -----------
            rpos_sb = constp.tile([16, 2048], f32, tag="rpos")
            # 4-way column-tiled: the four output slices run concurrently in
            # different 32-column groups of the PE array
            rp_all = psump.tile([128, 512], f32, tag="ps", name="rp_all")

            def rpos_mms(h):
                for n in range(4):
                    nc.tensor.matmul(
                        rp_all[32 * n:32 * n + 16, :],
                        lhsT=wbarT_sb[:, h, :],
                        rhs=posT_sb[:, h, n * 512:(n + 1) * 512],
                        start=(h == 0),
                        stop=(h == 1),
                        tile_position=(0, 32 * n),
                    )

            rpos_mms(0)

            # --------- conv matmuls, K-major, ordered by expected arrival ----
            # two M-waves: wave A (m0-4) is DMA-paced; wave B (m5-7) runs dense
            # afterwards while wave A normalizes -> most of the GN/act tail
            # overlaps wave B's matmuls.
            ypsum = [psump.tile([128, 512], f32, tag="ps", name=f"y{i}") for i in range(8)]
            k_order = [0, 1, 2, 3, 8, 10, 12, 14, 9, 11, 13, 15, 4, 5, 6, 7]

            def rhs_of(k):
                pr = pooled_rhs[k]
                if isinstance(pr, tuple):
                    _, h, r = pr
                    return pf[h][:, :, r, :]
                return pr[:]

            def conv_wave(ms):
                first, last = [], None
                for idx, k in enumerate(k_order):
                    rhs = rhs_of(k)
                    for m in ms:
                        inst = nc.tensor.matmul(
                            ypsum[m][:],
                            lhsT=convT_sb[:, k, m * 128:(m + 1) * 128],
                            rhs=rhs,
                            start=(idx == 0),
                            stop=(idx == 15),
                        )
                        if idx == 0:
                            first.append(inst)
                        last = inst
                return first, last

            WAVE_A = [0, 1, 2, 3, 4]
            WAVE_B1 = [5]
            WAVE_B2 = [6, 7]
            _, waveA_last = conv_wave(WAVE_A)
            rpos_mms(1)
            for n in range(4):
                nc.scalar.copy(rpos_sb[:, n * 512:(n + 1) * 512], rp_all[32 * n:32 * n + 16, :])
            nc.scalar.dma_start(rpos_d[:], rpos_sb[:])

            # --------- GroupNorm + ReLU + outputs, one round per wave --------
            # (groups of channel 128m+p are 2m + p//64: each 128-channel tile
            # is self-contained, so wave A normalizes while wave B matmuls)
            featv = feat_d[:].rearrange("(m p t) w -> m p (t w)", m=8, p=128, t=2)
            # featT2 physical layout: [q, h, k, (t, p)] so each transposed pair
            # lands with one contiguous copy; the roi matmul re-orders via AP.
            featT_sb = constp.tile([128, 2, 8, 256], bf16, tag="featT")
            rfeat_sb = constp.tile([16, 2048], f32, tag="rfeat")

            stats3 = workp.tile([128, 8, 3], f32, tag="stats3")
            nmu = workp.tile([128, 8], f32, tag="nmu")
            e2s = workp.tile([128, 8], f32, tag="e2s")
            musq = workp.tile([128, 8], f32, tag="musq")
            var = workp.tile([128, 8], f32, tag="var")
            std = workp.tile([128, 8], f32, tag="std")
            rstd = workp.tile([128, 8], f32, tag="rstd")
            scl = workp.tile([128, 8], f32, tag="scl")
            bias = workp.tile([128, 8], f32, tag="bias")

            def gn_stats(ms):
                for m in ms:
                    bnst = workp.tile([128, 6], f32, tag="bnst", bufs=2)
                    nc.vector.bn_stats(bnst[:], ypsum[m][:])
                    nc.vector.bn_aggr(stats3[:, m, 0:2], bnst[:])
                    nc.vector.tensor_mul(
                        stats3[:, m, 2:3], stats3[:, m, 0:1], stats3[:, m, 0:1]
                    )

            def gn_finish_from(ms, bcs):
                lo, hi = ms[0], ms[-1] + 1
                nm = hi - lo
                sl = slice(lo, hi)
                b3 = bcs[:, 0:3 * nm].rearrange("p (m s) -> p m s", m=nm)
                nc.vector.tensor_scalar_mul(nmu[:, sl], b3[:, :, 0], -1.0 / 64.0)
                nc.vector.tensor_add(e2s[:, sl], b3[:, :, 1], b3[:, :, 2])
                nc.vector.tensor_mul(musq[:, sl], nmu[:, sl], nmu[:, sl])
                nc.vector.tensor_scalar_mul(var[:, sl], e2s[:, sl], 1.0 / 64.0)
                nc.vector.tensor_sub(var[:, sl], var[:, sl], musq[:, sl])
                nc.scalar.activation(std[:, sl], var[:, sl], func=Sqrt, bias=eps_sb[:], scale=1.0)
                nc.vector.reciprocal(rstd[:, sl], std[:, sl])
                nc.vector.tensor_mul(scl[:, sl], gamma_sb[:, sl], rstd[:, sl])
                nc.vector.tensor_mul(bias[:, sl], nmu[:, sl], scl[:, sl])
                nc.vector.tensor_add(bias[:, sl], bias[:, sl], beta_sb[:, sl])

            def act_one(m, on_vector):
                featB = outp.tile([128, 512], bf16, tag="featB", bufs=6)
                if on_vector:
                    nc.vector.tensor_scalar(
                        featB[:], ypsum[m][:], scalar1=scl[:, m:m + 1],
                        scalar2=bias[:, m:m + 1],
                        op0=mybir.AluOpType.mult, op1=mybir.AluOpType.add,
                    )
                    nc.vector.tensor_scalar_max(featB[:], featB[:], 0.0)
                else:
                    nc.scalar.activation(
                        featB[:], ypsum[m][:], func=Relu,
                        bias=bias[:, m:m + 1], scale=scl[:, m:m + 1],
                    )
                # fp32 feat goes out via SWDGE cast-DMA (gpsimd is idle here)
                nc.gpsimd.dma_start(featv[m], featB[:])
                return featB

            def transpose_one(m, featB, copy_vec):
                fBv = featB[:].rearrange("p (t h w) -> p t h w", t=2, h=2, w=128)
                for h in range(2):
                    tp = psump.tile([128, 256], bf16, tag="ps", name=f"tp{m}_{h}")
                    for t in range(2):
                        nc.tensor.matmul(
                            tp[:, t * 128:(t + 1) * 128],
                            lhsT=fBv[:, t, h, :],
                            rhs=ident_sb[:],
                            is_transpose=True,
                            start=True,
                            stop=True,
                        )
                    if (h + (1 if copy_vec else 0)) % 2 == 0:
                        nc.scalar.copy(featT_sb[:, h, m, :], tp[:])
                    else:
                        nc.vector.tensor_copy(featT_sb[:, h, m, :], tp[:])

            def rfeat_pair(n0, cnt=2):
                # roi_feat slices packed into different 32-column groups
                rf = psump.tile([128, 512], f32, tag="ps", name=f"rf{n0}")
                for h in range(2):
                    for j in range(cnt):
                        n = n0 + j
                        rhs = featT_sb[:, h, 2 * n:2 * n + 2, :].rearrange(
                            "q k (t p) -> q k p t", t=2, p=128
                        )
                        nc.tensor.matmul(
                            rf[32 * j:32 * j + 16, :],
                            lhsT=wbarT_sb[:, h, :], rhs=rhs,
                            start=(h == 0), stop=(h == 1),
                            tile_position=(0, 32 * j),
                        )
                for j in range(cnt):
                    n = n0 + j
                    if j == 0:
                        nc.scalar.copy(rfeat_sb[:, n * 512:(n + 1) * 512], rf[32 * j:32 * j + 16, :])
                    else:
                        nc.vector.tensor_copy(rfeat_sb[:, n * 512:(n + 1) * 512], rf[32 * j:32 * j + 16, :])

            # wave A stats; cross-partition sums go through gpsimd so the PE
            # can start wave B with zero stall.  m5 joins GN round A: its stats
            # land right after wave B1 closes, so only m6/m7 remain in the
            # final round.
            gn_stats(WAVE_A)
            waveB1_first, waveB1_last = conv_wave(WAVE_B1)
            for inst in waveB1_first:
                tile.add_dep_helper(inst.ins, waveA_last.ins, reason="wave order")
            gn_stats(WAVE_B1)
            phi = workp.tile([128, 2, 18], f32, tag="phi")
            nc.vector.memset(phi[:], 0.0)
            nc.vector.tensor_copy(
                phi[0:64, 0, :], stats3[0:64, 0:6, :].rearrange("p m s -> p (m s)")
            )
            nc.vector.tensor_copy(
                phi[64:128, 1, :], stats3[64:128, 0:6, :].rearrange("p m s -> p (m s)")
            )
            phi2 = workp.tile([128, 2, 18], f32, tag="phi2")
            nc.gpsimd.partition_all_reduce(
                phi2[:], phi[:], 128, bass_isa.ReduceOp.add
            )
            waveB2_first, _ = conv_wave(WAVE_B2)
            for inst in waveB2_first:
                tile.add_dep_helper(inst.ins, waveB1_last.ins, reason="wave order")
            bcsA = workp.tile([128, 24], f32, tag="bcs", bufs=2)
            nc.vector.tensor_copy(bcsA[0:64, 0:18], phi2[0:64, 0, :])
            nc.vector.tensor_copy(bcsA[64:128, 0:18], phi2[64:128, 1, :])
            gn_finish_from([0, 1, 2, 3, 4, 5], bcsA)
            featBs = {}
            for m in [0, 1, 2, 3, 4, 5]:
                featBs[m] = act_one(m, on_vector=(m % 2 == 1))

            # round-B stats/PAR first in emission order: the vector engine
            # prioritizes the critical GN chain over round-A transpose copies
            gn_stats(WAVE_B2)
            phiB = workp.tile([128, 2, 6], f32, tag="phiB")
            nc.vector.memset(phiB[:], 0.0)
            nc.vector.tensor_copy(
                phiB[0:64, 0, :], stats3[0:64, 6:8, :].rearrange("p m s -> p (m s)")
            )
            nc.vector.tensor_copy(
                phiB[64:128, 1, :], stats3[64:128, 6:8, :].rearrange("p m s -> p (m s)")
            )
            phiB2 = workp.tile([128, 2, 6], f32, tag="phiB2")
            nc.gpsimd.partition_all_reduce(
                phiB2[:], phiB[:], 128, bass_isa.ReduceOp.add
            )
            bcsB = workp.tile([128, 24], f32, tag="bcs", bufs=2)
            nc.vector.tensor_copy(bcsB[0:64, 0:6], phiB2[0:64, 0, :])
            nc.vector.tensor_copy(bcsB[64:128, 0:6], phiB2[64:128, 1, :])
            gn_finish_from([6, 7], bcsB)

            # PE after wave B2: round-A transposes + roi slices, then m6/m7
            for m in [0, 1, 2, 3, 4, 5]:
                transpose_one(m, featBs[m], copy_vec=(m % 2 == 0))
                if m in (3, 5):
                    rfeat_pair(m - 3 if m == 3 else 2, 2 if m == 3 else 1)
            for m in [6, 7]:
                fB = act_one(m, on_vector=(m == 7))
                transpose_one(m, fB, copy_vec=(m % 2 == 0))
                if m == 7:
                    rfeat_pair(3, 1)
            nc.sync.dma_start(rfeat_d[:], rfeat_sb[:])

    nc.compile()
    return nc


def _get_nc():
    if "nc" not in _NC_CACHE:
        _NC_CACHE["nc"] = _build_nc()
    return _NC_CACHE["nc"]


def _roialign_wbar(rois):
    """Mean (over the 16x16 output pixels) RoIAlign weight vector per RoI.

    Numpy port of the reference's roialign_weights followed by mean over P.
    Returns [N, 256] float32.
    """
    rois = np.asarray(rois, np.float32)
    n = rois.shape[0]
    x1 = rois[:, 1] * SCALE - 0.5
    y1 = rois[:, 2] * SCALE - 0.5
    x2 = rois[:, 3] * SCALE - 0.5
    y2 = rois[:, 4] * SCALE - 0.5
    bw = (x2 - x1) / OUT_SIZE
    bh = (y2 - y1) / OUT_SIZE
    grid = (
        np.arange(OUT_SIZE, dtype=np.float32)[:, None]
        + (np.arange(RATIO, dtype=np.float32)[None, :] + 0.5) / RATIO
    )  # [O, r]
    ys = y1[:, None, None] + grid[None] * bh[:, None, None]  # [N, O, r]
    xs = x1[:, None, None] + grid[None] * bw[:, None, None]
    Y = np.broadcast_to(ys[:, :, None, :, None], (n, OUT_SIZE, OUT_SIZE, RATIO, RATIO))
    X = np.broadcast_to(xs[:, None, :, None, :], (n, OUT_SIZE, OUT_SIZE, RATIO, RATIO))
    valid = ((Y >= -1.0) & (Y <= HF) & (X >= -1.0) & (X <= WF)).astype(np.float32)
    y = np.maximum(Y, 0.0)
    x = np.maximum(X, 0.0)
    y0f = np.floor(y)
    x0f = np.floor(x)
    ye = y0f >= HF - 1
    xe = x0f >= WF - 1
    y0 = np.where(ye, HF - 1, y0f).astype(np.int32)
    y1i = np.where(ye, HF - 1, y0f + 1).astype(np.int32)
    x0 = np.where(xe, WF - 1, x0f).astype(np.int32)
    x1i = np.where(xe, WF - 1, x0f + 1).astype(np.int32)
    ly = np.where(ye, 0.0, y - y0f).astype(np.float32)
    lx = np.where(xe, 0.0, x - x0f).astype(np.float32)
    hy = 1.0 - ly
    hx = 1.0 - lx
    cnt = np.float32(RATIO * RATIO)
    w = np.stack([hy * hx, hy * lx, ly * hx, ly * lx], axis=-1) * (valid / cnt)[..., None]
    idx = np.stack(
        [y0 * WF + x0, y0 * WF + x1i, y1i * WF + x0, y1i * WF + x1i], axis=-1
    )
    wbar = np.zeros((n, HF * WF), np.float32)
    flat_idx = idx.reshape(n, -1)
    flat_w = (w / np.float32(OUT_SIZE * OUT_SIZE)).reshape(n, -1)
    np.add.at(wbar, (np.arange(n)[:, None], flat_idx), flat_w)
    return wbar


def _prep_in_maps(slow_feat, fast_feat, rois, pos, conv_w, gn_gamma, gn_beta):
    slow_feat = np.asarray(slow_feat, np.float32)
    fast_feat = np.asarray(fast_feat, np.float32)
    pos = np.asarray(pos, np.float32)
    conv_w = np.asarray(conv_w, np.float32)

    # 1/4 of the temporal mean folded into the conv weight
    convT = np.ascontiguousarray(conv_w.T * np.float32(0.25)).astype(BF16)
    posT = np.ascontiguousarray(pos.reshape(C_FEAT, P_PIX).T).astype(BF16)
    gamma8 = np.ascontiguousarray(
        np.asarray(gn_gamma, np.float32).reshape(8, 128).T
    )
    beta8 = np.ascontiguousarray(np.asarray(gn_beta, np.float32).reshape(8, 128).T)
    ident = np.eye(128, dtype=np.float32).astype(BF16)

    wbar = _roialign_wbar(rois)  # [128, 256]
    # rois are bucketed: N_PER per clip, sorted by batch index (static reshape
    # exactly as in the reference)
    wbarT_all = np.ascontiguousarray(
        wbar.reshape(B, N_PER, P_PIX).transpose(0, 2, 1)
    ).astype(BF16)  # [B, 256, 16]

    in_maps = []
    for b in range(B):
        in_maps.append(
            dict(
                slow=slow_feat[b].reshape(C_SLOW, T_SLOW * P_PIX).astype(BF16),
                fast=fast_feat[b].reshape(C_FAST, T_FAST * P_PIX).astype(BF16),
                convT=convT,
                posT=posT,
                wbarT=np.ascontiguousarray(wbarT_all[b]),
                gamma8=gamma8,
                beta8=beta8,
                ident=ident,
            )
        )
    return in_maps


def _ensure_ntff_hook():
    """Register the axon NTFF profile hook that the boot path skips when the
    image's antenv stub lacks axon_hooks. Test/profiling only."""
    try:
        from antenv.axon_hooks import get_axon_ntff_profile_hook  # noqa: F401
        return
    except ImportError:
        pass
    import types
    import antenv

    if "/root/.axon_site" not in sys.path:
        sys.path.insert(0, "/root/.axon_site")
    from trn_agent_boot.trn_boot import _ntff_profile_via_ctypes

    hook = _ntff_profile_via_ctypes("/opt/axon/libaxon_pjrt.so")
    mod = types.ModuleType("antenv.axon_hooks")
    mod.get_axon_ntff_profile_hook = lambda: hook
    mod.set_axon_ntff_profile_hook = lambda h: None
    sys.modules["antenv.axon_hooks"] = mod
    antenv.axon_hooks = mod

    # artifact upload has no bucket in this container; neuter it
    from concourse import bass_utils

    bass_utils.upload_artifacts = lambda tmpdir: tmpdir


def _run(in_maps, trace=False):
    from concourse.bass_utils import run_bass_kernel_spmd

    if trace:
        _ensure_ntff_hook()
    nc = _get_nc()
    res = run_bass_kernel_spmd(nc, in_maps, core_ids=list(range(B)), trace=trace)
    _NC_CACHE["last_res"] = res
    return res


def _assemble(res):
    feat = np.stack(
        [res.results[b]["feat"].reshape(C_FEAT, HF, WF) for b in range(B)]
    )
    roi_feat = np.stack([res.results[b]["roi_feat"] for b in range(B)])
    roi_pos = np.stack([res.results[b]["roi_pos"] for b in range(B)])
    return feat, roi_feat, roi_pos


def kernel(slow_feat, fast_feat, rois, pos, conv_w, gn_gamma, gn_beta):
    in_maps = _prep_in_maps(slow_feat, fast_feat, rois, pos, conv_w, gn_gamma, gn_beta)
    res = _run(in_maps, trace=False)
    return _assemble(res)


def kernel_traced(slow_feat, fast_feat, rois, pos, conv_w, gn_gamma, gn_beta):
    """Same as kernel() but captures a neuron-profile trace; returns
    (outputs, exec_time_ns)."""
    in_maps = _prep_in_maps(slow_feat, fast_feat, rois, pos, conv_w, gn_gamma, gn_beta)
    res = _run(in_maps, trace=True)
    return _assemble(res), res.exec_time_ns
